# revision 64
# baseline (speedup 1.0000x reference)
"""Distributed MultiHeadAttention + residual + LayerNorm for 8 TRN2 NeuronCores.

Problem: B=2, S=2048, E=1024, H=16, Dh=64 (fp32 in/out).
Sharding: core c = (batch b=c//4, head-group g=c%4); each core computes 4 heads
for its batch. Output projection is row-sharded on the att dim; partials are
combined with per-sblock 4-rank ReduceScatter, then residual+LayerNorm happen
on each core's own row shard.

Single fused pipeline per 512-row sblock:
  [qkv-proj(sb) -> attention(sb) -> (deferred) out-proj(sb-1) -> RS(sb-1)]
The out-projection of block sb is emitted after attention(sb+1) so its
softmax-normalize chain (recip -> partition_broadcast -> mul) never idles the
PE into a low p-state; the ReduceScatter chain runs continuously behind
compute and only the last block's RS + LayerNorm are exposed in the tail.

fp8e4(DoubleRow) matmuls (2 k-tiles per pass = 2x PE throughput) are used for
the QKV projections (x and weights both e4m3; weights pre-scaled by 16 on the
host to stay clear of e4m3 subnormals, de-scaled at PSUM evacuation) and for
the output projection (att and wo both carried at 16x in fp8; both dg halves
contract in one DoubleRow matmul; the 1/256 folds into the evacuation).

Scores are computed transposed (scoresT[t, s]) in bf16 so softmax row sums
come out of the ctx matmul for free via a ones-column appended to V; the
1/sqrt(Dh) scale folds into the exp activation (scale=0.125). The flattened
(dg, tile) loop runs with one tile of lookahead (scores(t+1) before ctx(t))
so PE and the exp-saturated ACT engine pipeline instead of ping-ponging.
Engine placement: exp + proj evacuations on ACT, softmax-normalize + out-proj
evacuations + LN stats on DVE, broadcasts/adds/collectives on Pool (which
cannot touch PSUM on real HW), x loads split across SP/ACT DMA queues in
arrival-priority order. Scheduler-only no_sync_barrier fences pin the LN
chains to the tail so their collective waits never head-of-line block the
pipeline.

Mask handling: tiles are classified on the host from the actual mask input:
fully-masked tiles are skipped, each computed tile gets a column window
[c_lo, 512) excluding fully-masked columns, and a narrow band [b0, b1) where
exp() output is multiplied by a 0/1 keep matrix. The causal structure is
exploited without being hardcoded. LayerNorm takes a fused path when
ln_g==1/ln_b==0 (checked on the host, part of the build cache key).
"""
import sys

if "/opt/trn_rl_repo" not in sys.path:
    sys.path.insert(0, "/opt/trn_rl_repo")

from contextlib import ExitStack

import numpy as np
import ml_dtypes

import concourse.bacc as bacc
import concourse.mybir as mybir
import concourse.tile as tile
from concourse.bass_utils import run_bass_kernel_spmd

BF16 = ml_dtypes.bfloat16
FP8 = ml_dtypes.float8_e4m3fn
F32 = mybir.dt.float32
BF = mybir.dt.bfloat16
F8 = mybir.dt.float8e4

B, S, E, H = 2, 2048, 1024, 16
Dh = E // H
LN_EPS = 1e-5
N_CORES = 8
HL = 4            # local heads per core
GROUPS = [[0, 1, 2, 3], [4, 5, 6, 7]]
NS = 4            # s blocks of 512 (query positions)
SBLK = 512
NT = 16           # t chunks of 128 (key positions)
TBLK = 128
NE = 8            # e chunks of 128 (contraction over E)
NEP = 4           # e chunk pairs (DoubleRow)
WSCALE = 16.0     # host-side weight scale (de-scaled at evacuation)
Act = mybir.ActivationFunctionType
Alu = mybir.AluOpType
DR = mybir.MatmulPerfMode.DoubleRow

_BUILD_CACHE = {}


def _dn128(x):
    return (x // 128) * 128


def _up128(x):
    return -(-x // 128) * 128


def classify_mask(mask):
    """Host-side tile classification from the actual mask input.

    Returns a structure tuple:
      tiles[sb] = list of (j, c_lo, band) with band = (b0, b1, keep_idx) or None
      kw = keep tensor width
    Shared across batches (union), so one SPMD program serves all cores.
    """
    tiles = []
    mult_list = []   # (sb, j, b0, b1)
    for sb in range(NS):
        lst = []
        for j in range(NT):
            # region[b] = mask[b, s-rows, t-cols]; col c of tile = s index
            reg = mask[:, sb * SBLK:(sb + 1) * SBLK, j * TBLK:(j + 1) * TBLK]
            keep_any_col = (~reg).any(axis=2).any(axis=0)     # [SBLK] per s col
            if not keep_any_col.any():
                continue  # fully masked in every batch
            c_lo = _dn128(int(np.argmax(keep_any_col)))
            # prefix [0, c_lo) must be fully masked in all batches (guaranteed
            # since keep_any_col is False there)
            masked_any_col = reg.any(axis=2).any(axis=0)      # [SBLK]
            masked_any_col[:c_lo] = False
            if masked_any_col.any():
                nz = np.nonzero(masked_any_col)[0]
                b0 = max(c_lo, _dn128(int(nz[0])))
                b1 = min(SBLK, _up128(int(nz[-1]) + 1))
                lst.append((j, c_lo, (b0, b1, len(mult_list))))
                mult_list.append((sb, j, b0, b1))
            else:
                lst.append((j, c_lo, None))
        # widest window first so the ctx accumulation group starts with a
        # full-bank clear
        lst.sort(key=lambda t: (t[1], t[0]))
        tiles.append(tuple(lst))
    kw = max([b1 - b0 for (_, _, b0, b1) in mult_list], default=128)
    return tuple(tiles), tuple(mult_list), kw


def build(tiles, mult_list, kw, trivial_gb=False):
    nm = max(1, len(mult_list))
    nc = bacc.Bacc("TRN2", num_devices=N_CORES)

    # ---- I/O -------------------------------------------------------------
    qT_d = nc.dram_tensor("qT", [E, S], F8, kind="ExternalInput")
    kT_d = nc.dram_tensor("kT", [E, S], F8, kind="ExternalInput")
    vT_d = nc.dram_tensor("vT", [E, S], F8, kind="ExternalInput")
    # weights pre-arranged to [128, NEP, 2, 256] (flattened) on the host
    wq_d = nc.dram_tensor("wq", [128, NEP * 2 * 256], F8, kind="ExternalInput")
    wk_d = nc.dram_tensor("wk", [128, NEP * 2 * 256], F8, kind="ExternalInput")
    wv_d = nc.dram_tensor("wv", [128, NEP * 2 * 256], F8, kind="ExternalInput")
    qb_d = nc.dram_tensor("qb", [128, 2], F32, kind="ExternalInput")
    kb_d = nc.dram_tensor("kb", [128, 2], F32, kind="ExternalInput")
    # keep is host-packed p-major ([128, nm*kw]) for 2KB DMA descriptors
    keep_d = nc.dram_tensor("keep", [TBLK, nm * kw], BF, kind="ExternalInput")
    wo_d = nc.dram_tensor("wo", [256, E], F8, kind="ExternalInput")
    resid_d = nc.dram_tensor("resid", [512, E], BF, kind="ExternalInput")
    lng_d = nc.dram_tensor("lng", [1, E], F32, kind="ExternalInput")
    lnb_d = nc.dram_tensor("lnb", [1, E], F32, kind="ExternalInput")
    out_d = nc.dram_tensor("out", [512, E], F32, kind="ExternalOutput")

    rs_in = nc.dram_tensor("rs_in", [S, E], BF, kind="Internal")
    rs_out = nc.dram_tensor("rs_out", [512, E], BF, kind="Internal")

    with tile.TileContext(nc) as tc, ExitStack() as ctx:
        # ---- persistent SBUF tiles --------------------------------------
        persist = ctx.enter_context(tc.tile_pool(name="persist", bufs=1))
        # x split into sb0 / sb1 / sb2+3 tiles so attention(0) can start as
        # soon as the first 1.5MB lands instead of after the full 6MB
        x_qs = [persist.tile([128, NE, SBLK], F8, name="x_q0"),
                persist.tile([128, NE, SBLK], F8, name="x_q1"),
                persist.tile([128, NE, 2 * SBLK], F8, name="x_q23")]
        x_ks = [persist.tile([128, NE, SBLK], F8, name="x_k0"),
                persist.tile([128, NE, SBLK], F8, name="x_k1"),
                persist.tile([128, NE, 2 * SBLK], F8, name="x_k23")]
        x_vs = [persist.tile([128, NE, SBLK], F8, name="x_v0"),
                persist.tile([128, NE, SBLK], F8, name="x_v1"),
                persist.tile([128, NE, 2 * SBLK], F8, name="x_v23")]
        q_all = persist.tile([128, 2, S], BF, name="q_all")   # [d-pair, dg, s]
        k_all = persist.tile([128, 2, S], BF, name="k_all")
        v_all = persist.tile([128, NT, HL, 65], BF, name="v_all")  # [t, j, h, d|1]
        att_sb = persist.tile([128, 2, S], F8, name="att_sb")  # [d-pair, dg, s]
        keep_sb = persist.tile([128, nm, kw], BF, name="keep_sb")
        qb_sb = persist.tile([128, 2], F32, name="qb_sb")
        kb_sb = persist.tile([128, 2], F32, name="kb_sb")
        wq_sb = persist.tile([128, NEP, 2, 256], F8, name="wq_sb")
        wk_sb = persist.tile([128, NEP, 2, 256], F8, name="wk_sb")
        wv_sb = persist.tile([128, NEP, 2, 256], F8, name="wv_sb")
        g_bc = persist.tile([128, E], F32, name="g_bc")
        b_bc = persist.tile([128, E], F32, name="b_bc")
        magic = persist.tile([128, 1], mybir.dt.uint32, name="magic")
        wo_sb = persist.tile([128, 2, E], F8, name="wo_sb")    # [d-pair, dg, e]
        resid_sb = persist.tile([128, NS, E], BF, name="resid_sb")

        nc.vector.memset(magic, 0x5F3759DF)
        # ones column for the row-sum trick
        nc.vector.memset(v_all[:, :, :, 64:65], 1.0)

        # ---- upfront DMA loads --------------------------------------------
        # ALL on the SP queue: ACT must issue no bulk DMAs (a full HWDGE ring
        # blocks its SEQ for >10us, delaying the first evacuations/exps) and
        # SP has no compute until the first rs_in DMAs ~30us in. The DMA
        # engine pool drains FIFO, so issue order == need order: each proj's
        # weight immediately before its x slice (k first: it heads the PE
        # stream), then keep (first band exps), x1, wo (outproj(0)), x23.
        # resid is loaded per-sblock inside proj_rs.
        XCOLS = [slice(0, SBLK), slice(SBLK, 2 * SBLK), slice(2 * SBLK, S)]
        nc.sync.dma_start(out=wk_sb,
                          in_=wk_d.rearrange("p (a b c) -> p a b c", a=NEP, b=2))
        nc.sync.dma_start(out=kb_sb, in_=kb_d[:, :])
        nc.sync.dma_start(
            out=x_ks[0], in_=kT_d[:, XCOLS[0]].rearrange("(c p) s -> p c s", p=128))
        nc.sync.dma_start(out=wq_sb,
                          in_=wq_d.rearrange("p (a b c) -> p a b c", a=NEP, b=2))
        nc.sync.dma_start(out=qb_sb, in_=qb_d[:, :])
        nc.sync.dma_start(
            out=x_qs[0], in_=qT_d[:, XCOLS[0]].rearrange("(c p) s -> p c s", p=128))
        nc.sync.dma_start(out=wv_sb,
                          in_=wv_d.rearrange("p (a b c) -> p a b c", a=NEP, b=2))
        nc.sync.dma_start(
            out=x_vs[0], in_=vT_d[:, XCOLS[0]].rearrange("(c p) s -> p c s", p=128))
        nc.sync.dma_start(out=keep_sb,
                          in_=keep_d.rearrange("p (m s) -> p m s", m=nm))
        for gi in (1, 2):
            nc.sync.dma_start(
                out=x_qs[gi],
                in_=qT_d[:, XCOLS[gi]].rearrange("(c p) s -> p c s", p=128))
            nc.sync.dma_start(
                out=x_ks[gi],
                in_=kT_d[:, XCOLS[gi]].rearrange("(c p) s -> p c s", p=128))
            nc.sync.dma_start(
                out=x_vs[gi],
                in_=vT_d[:, XCOLS[gi]].rearrange("(c p) s -> p c s", p=128))
            if gi == 1:
                nc.sync.dma_start(out=wo_sb,
                                  in_=wo_d.rearrange("(c p) d -> p c d", p=128))

        # ---- pools -------------------------------------------------------
        pp_pool = ctx.enter_context(tc.tile_pool(name="pp", bufs=2, space="PSUM"))
        sc_pool = ctx.enter_context(tc.tile_pool(name="sc", bufs=2, space="PSUM"))
        ctx_pool = ctx.enter_context(tc.tile_pool(name="ctxp", bufs=2, space="PSUM"))
        probs_pool = ctx.enter_context(tc.tile_pool(name="probs", bufs=6))
        small = ctx.enter_context(tc.tile_pool(name="small", bufs=4))
        work = ctx.enter_context(tc.tile_pool(name="work", bufs=4))
        lnp = ctx.enter_context(tc.tile_pool(name="lnp", bufs=2))

        INV_W = 1.0 / WSCALE

        def xgrp(sb):
            # (tile-group index, column offset within the group) for sblock
            return (sb, 0) if sb < 2 else (2, (sb - 2) * SBLK)

        def qk_proj(sb, xts, w_sb, bias_sb, dst):
            gi, off = xgrp(sb)
            xs = slice(off, off + SBLK)
            ss = slice(sb * SBLK, (sb + 1) * SBLK)
            for dg in range(2):
                ps = pp_pool.tile([128, SBLK], F32, name="ps", tag="pp")
                for ep in range(NEP):
                    nc.tensor.matmul(
                        ps,
                        w_sb[:, ep, :, dg * 128:(dg + 1) * 128],
                        xts[gi][:, 2 * ep:2 * ep + 2, xs],
                        start=(ep == 0), stop=(ep == NEP - 1),
                        perf_mode=DR,
                    )
                # evacuation on ACT: fused 1/WSCALE descale + bias add
                nc.scalar.activation(dst[:, dg, ss], ps, Act.Identity,
                                     bias=bias_sb[:, dg:dg + 1], scale=INV_W)

        def v_proj(sb):
            gi, off = xgrp(sb)
            # v[t, (h d)] for the 4 t-tiles of this sblock, two t-tiles per
            # PSUM bank
            for jj in range(2):
                j0 = 4 * sb + 2 * jj
                psv = pp_pool.tile([128, SBLK], F32, name="ps", tag="pp")
                for tl in range(2):
                    toff = off + (2 * jj + tl) * TBLK
                    ts = slice(toff, toff + TBLK)
                    for ep in range(NEP):
                        nc.tensor.matmul(
                            psv[:, tl * 256:(tl + 1) * 256],
                            x_vs[gi][:, 2 * ep:2 * ep + 2, ts],
                            wv_sb[:, ep, :, :],
                            start=(ep == 0), stop=(ep == NEP - 1),
                            perf_mode=DR,
                        )
                # v gets an extra x16 (cancelling 1/WSCALE exactly): att_sb
                # is carried at 16x so its fp8 quantization error is relative,
                # and the 1/(16*16) is folded into the outproj evacuation
                nc.scalar.activation(
                    v_all[:, j0:j0 + 2, :, 0:64],
                    psv.rearrange("p (j h d) -> p j h d", j=2, h=HL),
                    Act.Identity, scale=1.0)

        def attention(sb):
            tlist = tiles[sb]

            def normalize(dg, cA, cB):
                # recip from an SBUF copy of the PSUM ones-row (the custom-DVE
                # recip op reads garbage from PSUM on real HW), broadcast on
                # Pool; this chain gates outproj. For the LAST block (whose
                # chain is fully exposed in the tail) the sums copies run in
                # parallel on ACT (idle after the final exp) and DVE, and the
                # normalize muls split into column chunks so the outproj
                # chunks emitted right after can pipeline with them.
                last_blk = sb == NS - 1 and dg == 1
                for i, cx in ((0, cA), (1, cB)):
                    sums = small.tile([1, SBLK], F32, name="sums", tag="sums")
                    ceng = nc.scalar if (last_blk and i == 0) else nc.vector
                    if ceng is nc.scalar:
                        ceng.activation(sums, cx[64:65, :], Act.Identity)
                    else:
                        ceng.tensor_copy(sums, cx[64:65, :])
                    recip = small.tile([1, SBLK], F32, name="recip", tag="recip")
                    nc.vector.reciprocal_approx_fast(recip, sums)
                    bc = work.tile([64, SBLK], F32, name="bc", tag="bc")
                    nc.gpsimd.partition_broadcast(bc, recip, channels=64)
                    ncols = 4 if last_blk else 1
                    for cchunk in range(ncols):
                        colw = SBLK // ncols
                        c0 = cchunk * colw
                        nc.vector.tensor_mul(
                            att_sb[64 * i:64 * i + 64, dg,
                                   sb * SBLK + c0:sb * SBLK + c0 + colw],
                            cx[0:64, c0:c0 + colw], bc[:, c0:c0 + colw])

            # single flattened (dg, tile) stream with one tile of lookahead
            # ACROSS the dg boundary so ACT never drains mid-block
            cAB = {}
            pend = None

            def emit_ctx(p):
                _dg, _j, _cs, _p2, _first, _last = p
                cA, cB = cAB[_dg]
                nc.tensor.matmul(cA[:, _cs], v_all[:, _j, 2 * _dg, :],
                                 _p2[:, 0, _cs], start=_first, stop=_last)
                nc.tensor.matmul(cB[:, _cs], v_all[:, _j, 2 * _dg + 1, :],
                                 _p2[:, 1, _cs], start=_first, stop=_last)
                if _last:
                    normalize(_dg, cA, cB)

            for dg in range(2):
                for idx, (j, c_lo, band) in enumerate(tlist):
                    first, last = idx == 0, idx == len(tlist) - 1
                    if first:
                        cAB[dg] = (
                            ctx_pool.tile([65, SBLK], F32, name="cA", tag="ctx"),
                            ctx_pool.tile([65, SBLK], F32, name="cB", tag="ctx"))
                    # two-bank scores tile: head A in half 0, head B in half 1
                    sc2 = sc_pool.tile([128, 2, SBLK], F32, name="sc2", tag="sc")
                    ts = slice(j * TBLK, (j + 1) * TBLK)
                    ss = slice(sb * SBLK + c_lo, (sb + 1) * SBLK)
                    cs = slice(c_lo, SBLK)
                    nc.tensor.matmul(sc2[:, 0, cs], k_all[0:64, dg, ts],
                                     q_all[0:64, dg, ss], start=True, stop=True)
                    nc.tensor.matmul(sc2[:, 1, cs], k_all[64:128, dg, ts],
                                     q_all[64:128, dg, ss], start=True, stop=True)
                    p2 = probs_pool.tile([128, 2, SBLK], BF, name="p2", tag="pr")
                    nc.scalar.activation(p2[:, :, cs], sc2[:, :, cs], Act.Exp,
                                         scale=0.125)
                    if band is not None:
                        b0, b1, mi = band
                        bs_ = slice(b0, b1)
                        keep_b = keep_sb[:, mi:mi + 1, 0:b1 - b0].to_broadcast(
                            [128, 2, b1 - b0])
                        nc.vector.tensor_mul(p2[:, :, bs_], p2[:, :, bs_], keep_b)
                    if pend is not None:
                        emit_ctx(pend)
                    pend = (dg, j, cs, p2, first, last)
            emit_ctx(pend)

        def proj_rs(sb):
            # out projection partials + ReduceScatter for this sblock
            if sb == 1 and not trivial_gb:
                # LN consts, needed first at post_ln(0) (emitted two sblocks
                # later): issued here to stay clear of the x-chunk burst
                nc.gpsimd.dma_start(out=g_bc, in_=lng_d[0:1, :].to_broadcast([128, E]))
                nc.gpsimd.dma_start(out=b_bc, in_=lnb_d[0:1, :].to_broadcast([128, E]))
            # residual rows for this sblock's LN, prefetched well before use
            nc.sync.dma_start(out=resid_sb[:, sb, :],
                              in_=resid_d[sb * 128:(sb + 1) * 128, :])
            for sc in range(4):   # 128-row chunks within sblock
                srow = sb * 4 + sc
                row = slice(srow * 128, (srow + 1) * 128)
                pcopy = work.tile([128, E], BF, name="pcopy", tag="pcopy")
                for eo in range(2):
                    pp = pp_pool.tile([128, SBLK], F32, name="ps", tag="pp")
                    # both dg halves contract in ONE DoubleRow fp8 matmul
                    nc.tensor.matmul(
                        pp,
                        att_sb[:, :, row],
                        wo_sb[:, :, eo * SBLK:(eo + 1) * SBLK],
                        start=True, stop=True,
                        perf_mode=DR,
                    )
                    # 1/256 undoes the x16 on att_sb and the x16 on wo.
                    # For the last sblock ACT is past its final exp, so
                    # splitting the evacuations ACT/DVE halves the stagger
                    # on the rs_in(3) critical path.
                    if sb == NS - 1 and eo == 0:
                        nc.scalar.activation(pcopy[:, 0:SBLK], pp,
                                             Act.Identity, scale=1.0 / 256.0)
                    else:
                        nc.vector.tensor_scalar_mul(
                            pcopy[:, eo * SBLK:(eo + 1) * SBLK], pp, 1.0 / 256.0)
                nc.sync.dma_start(out=rs_in[row, :], in_=pcopy)
            nc.gpsimd.collective_compute(
                "ReduceScatter", Alu.add,
                ins=[rs_in[sb * SBLK:(sb + 1) * SBLK, :]],
                outs=[rs_out[sb * 128:(sb + 1) * 128, :]],
                replica_groups=GROUPS,
            )

        def post_ln(sb, act_rsqrt=False):
            # residual + LN on own 128 rows of this sblock (runs one sblock
            # behind the RS so its waits never head-of-line block the queues)
            pchunk = lnp.tile([128, E], BF, name="pchunk", tag="pchunk")
            x_t = lnp.tile([128, E], F32, name="x_t", tag="x_t")
            stats = small.tile([128, 2, 6], F32, name="stats", tag="stats")
            for h in range(2):
                hs = slice(h * 512, (h + 1) * 512)
                deng = nc.sync if h == 0 else nc.scalar
                deng.dma_start(out=pchunk[:, hs],
                               in_=rs_out[sb * 128:(sb + 1) * 128, hs])
                eng = nc.gpsimd if h == 0 else nc.vector
                eng.tensor_add(x_t[:, hs], resid_sb[:, sb, hs], pchunk[:, hs])
                nc.vector.bn_stats(stats[:, h, :], x_t[:, hs])
            mv = small.tile([128, 2], F32, name="mv", tag="mv")
            nc.vector.bn_aggr(mv, stats)
            # rstd = rsqrt(var + eps) on DVE (bit-trick seed + 2 Newton iters)
            # so ACT never leaves the exp table set
            U32 = mybir.dt.uint32
            ws = small.tile([128, 1], F32, name="ws", tag="ws")
            nc.vector.tensor_scalar_add(ws, mv[:, 1:2], LN_EPS)
            rstd = small.tile([128, 1], F32, name="rstd", tag="rstd")
            if act_rsqrt:
                # tail LNs run after the last exp: the ACT table switch is
                # free to take there; Sqrt+reciprocal replaces the Newton
                # chain (Rsqrt itself is blocked for accuracy)
                sq = small.tile([128, 1], F32, name="sq", tag="sq")
                nc.scalar.activation(sq, ws, Act.Sqrt)
                nc.vector.reciprocal(rstd, sq)
            else:
                hbits = small.tile([128, 1], U32, name="hbits", tag="hbits")
                nc.vector.tensor_scalar(hbits, ws.bitcast(U32), 1, None,
                                        op0=Alu.logical_shift_right)
                nc.vector.scalar_tensor_tensor(
                    rstd.bitcast(U32), magic, 0, hbits, op0=Alu.bypass,
                    op1=Alu.subtract)
                nt = small.tile([128, 1], F32, name="nt", tag="nt")
                for _ in range(2):
                    nc.vector.tensor_mul(nt, ws, rstd)
                    nc.vector.tensor_mul(nt, nt, rstd)
                    nc.vector.tensor_scalar(nt, nt, -0.5, 1.5, op0=Alu.mult,
                                            op1=Alu.add)
                    nc.vector.tensor_mul(rstd, rstd, nt)
            o_t = lnp.tile([128, E], F32, name="o_t", tag="o_t")
            if trivial_gb:
                # ln_g==1, ln_b==0 (checked on the host): a single
                # (x-mu)*rstd per half, with the output DMA overlapping the
                # second half's compute
                for h in range(2):
                    hs = slice(h * 512, (h + 1) * 512)
                    nc.vector.tensor_scalar(o_t[:, hs], x_t[:, hs],
                                            mv[:, 0:1], rstd,
                                            op0=Alu.subtract, op1=Alu.mult)
                    eng = nc.sync if h == 0 else nc.scalar
                    eng.dma_start(out=out_d[sb * 128:(sb + 1) * 128, hs],
                                  in_=o_t[:, hs])
            else:
                y_t = lnp.tile([128, E], F32, name="y_t", tag="y_t")
                nc.vector.scalar_tensor_tensor(
                    y_t, x_t, mv[:, 0:1], g_bc, op0=Alu.subtract, op1=Alu.mult)
                nc.vector.scalar_tensor_tensor(
                    o_t, y_t, rstd, b_bc, op0=Alu.mult, op1=Alu.add)
                nc.sync.dma_start(out=out_d[sb * 128:(sb + 1) * 128, :], in_=o_t)

        # Pipeline: proj(sb+1) is emitted between attention(sb) and
        # outproj(sb) so the PE has work while the softmax-normalize chain
        # (recip -> broadcast -> mul) completes, instead of idling into a
        # low p-state.
        def kv_proj(sb):
            qk_proj(sb, x_ks, wk_sb, kb_sb, k_all)
            v_proj(sb)

        # Attention visit order [1, 2, 3, 0]: the ReduceScatter chain starts
        # after the second-smallest block (~32us) and stays continuous —
        # each rs_in arrives just as the previous collective finishes — and
        # compute ends on the smallest block, so only one RS+LN is exposed
        # in the tail. k/v projections still build incrementally (attention
        # (sb) needs k/v of every block up to sb).
        # proj_rs(sb) is deferred one block (emitted after attention(sb+1))
        # so the softmax-normalize chain of block sb never stalls the PE:
        # attention(sb+1) fills that window.
        kv_proj(0)
        qk_proj(0, x_qs, wq_sb, qb_sb, q_all)
        for sb in range(NS):
            attention(sb)
            if sb >= 1:
                proj_rs(sb - 1)
            if sb + 1 < NS:
                kv_proj(sb + 1)
                qk_proj(sb + 1, x_qs, wq_sb, qb_sb, q_all)
            if sb == NS - 1:
                proj_rs(sb)
            if sb == 2:
                # scheduler-only fences: without them the Tile scheduler
                # hoists the LN chains (whose first op waits on a
                # ReduceScatter) into the middle of the pipeline,
                # head-of-line blocking DVE/SP behind collective waits.
                tc.no_sync_barrier()
                post_ln(0)
        for sb in range(1, NS):
            # one fence per LN block: the scheduler otherwise reorders the
            # rs_out->pchunk DMAs across blocks, head-of-line blocking an
            # already-satisfied LN behind the last collective
            tc.no_sync_barrier()
            post_ln(sb, act_rsqrt=True)

    nc.finalize()
    return nc


def _prep_core(inputs, b, g):
    heads = slice(HL * g, HL * (g + 1))
    query = np.asarray(inputs["query"][b], np.float32)
    key = np.asarray(inputs["key"][b], np.float32)
    value = np.asarray(inputs["value"][b], np.float32)
    Wq_w = np.asarray(inputs["Wq_w"], np.float32)
    Wk_w = np.asarray(inputs["Wk_w"], np.float32)
    Wv_w = np.asarray(inputs["Wv_w"], np.float32)
    Wq_b = np.asarray(inputs["Wq_b"], np.float32)
    Wk_b = np.asarray(inputs["Wk_b"], np.float32)
    Wv_b = np.asarray(inputs["Wv_b"], np.float32)
    out_w = np.asarray(inputs["out_w"], np.float32)
    out_b = np.asarray(inputs["out_b"], np.float32)

    def packb(t):  # [4, 64] -> [128, 2] pair-major
        return np.ascontiguousarray(
            t.reshape(2, 2, Dh).transpose(1, 2, 0).reshape(128, 2))

    def packw(Wh):  # [4, Dh, E] head-major -> [128, NEP*2*256] DR layout
        w = Wh.reshape(256, E).T * WSCALE            # [E, 256]
        w = w.reshape(NEP, 2, 128, 256).transpose(2, 0, 1, 3)
        return np.ascontiguousarray(w.reshape(128, NEP * 2 * 256)).astype(FP8)

    d = {
        "qT": np.ascontiguousarray(query.T).astype(FP8),
        "kT": np.ascontiguousarray(key.T).astype(FP8),
        "vT": np.ascontiguousarray(value.T).astype(FP8),
        "wq": packw(Wq_w[heads]),
        "wk": packw(Wk_w[heads]),
        "wv": packw(Wv_w[heads]),
        "qb": packb(Wq_b[heads]),
        "kb": packb(Wk_b[heads]),
        "wo": np.ascontiguousarray(
            out_w[:, 256 * g:256 * (g + 1)].T * WSCALE).astype(FP8),
        "lng": np.asarray(inputs["ln_g"], np.float32).reshape(1, E).copy(),
        "lnb": np.asarray(inputs["ln_b"], np.float32).reshape(1, E).copy(),
    }
    const = out_b + Wv_b.reshape(E) @ out_w.T
    rows = query.reshape(NS, 4, 128, E)[:, g, :, :].reshape(512, E)
    d["resid"] = np.ascontiguousarray(rows + const[None, :]).astype(BF16)
    return d


def _prep_keep(mask, mult_list, kw, b):
    """Keep matrix packed p-major: [TBLK, nm*kw] bf16."""
    nm = max(1, len(mult_list))
    keep = np.zeros((TBLK, nm, kw), np.float32)
    for mi, (sb, j, b0, b1) in enumerate(mult_list):
        reg = mask[b, sb * SBLK + b0:sb * SBLK + b1,
                   j * TBLK:(j + 1) * TBLK]
        keep[:, mi, 0:b1 - b0] = (~reg).T.astype(np.float32)
    return np.ascontiguousarray(keep.reshape(TBLK, nm * kw)).astype(BF16)


def kernel(**inputs):
    mask = np.asarray(inputs["mask"], bool)
    tiles, mult_list, kw = classify_mask(mask)
    trivial_gb = bool(np.all(np.asarray(inputs["ln_g"]) == 1.0)
                      and np.all(np.asarray(inputs["ln_b"]) == 0.0))
    key_struct = (tiles, mult_list, kw, trivial_gb)
    if key_struct not in _BUILD_CACHE:
        _BUILD_CACHE[key_struct] = build(tiles, mult_list, kw, trivial_gb)
    nc = _BUILD_CACHE[key_struct]

    in_maps = []
    for c in range(N_CORES):
        b, g = c // 4, c % 4
        d = _prep_core(inputs, b, g)
        d["keep"] = _prep_keep(mask, mult_list, kw, b)
        in_maps.append(d)

    res = run_bass_kernel_spmd(nc, in_maps, core_ids=list(range(N_CORES)))

    out = np.empty((B, S, E), np.float32)
    for c in range(N_CORES):
        b, g = c // 4, c % 4
        o = res.results[c]["out"]  # [512, E]
        for sb in range(NS):
            out[b, sb * SBLK + 128 * g: sb * SBLK + 128 * (g + 1), :] = \
                o[sb * 128:(sb + 1) * 128, :]
    return out


# revision 67
# speedup vs baseline: 1.0071x; 1.0071x over previous
"""Distributed MultiHeadAttention + residual + LayerNorm for 8 TRN2 NeuronCores.

Problem: B=2, S=2048, E=1024, H=16, Dh=64 (fp32 in/out).
Sharding: core c = (batch b=c//4, head-group g=c%4); each core computes 4 heads
for its batch. Output projection is row-sharded on the att dim; partials are
combined with per-sblock 4-rank ReduceScatter, then residual+LayerNorm happen
on each core's own row shard.

Single fused pipeline per 512-row sblock:
  [qkv-proj(sb) -> attention(sb) -> (deferred) out-proj(sb-1) -> RS(sb-1)]
The out-projection of block sb is emitted after attention(sb+1) so its
softmax-normalize chain (recip -> partition_broadcast -> mul) never idles the
PE into a low p-state; the ReduceScatter chain runs continuously behind
compute and only the last block's RS + LayerNorm are exposed in the tail.

fp8e4(DoubleRow) matmuls (2 k-tiles per pass = 2x PE throughput) are used for
the QKV projections (x and weights both e4m3; weights pre-scaled by 16 on the
host to stay clear of e4m3 subnormals, de-scaled at PSUM evacuation) and for
the output projection (att and wo both carried at 16x in fp8; both dg halves
contract in one DoubleRow matmul; the 1/256 folds into the evacuation).

Scores are computed transposed (scoresT[t, s]) in bf16 so softmax row sums
come out of the ctx matmul for free via a ones-column appended to V; the
1/sqrt(Dh) scale folds into the exp activation (scale=0.125). The flattened
(dg, tile) loop runs with one tile of lookahead (scores(t+1) before ctx(t))
so PE and the exp-saturated ACT engine pipeline instead of ping-ponging.
Engine placement: exp + proj evacuations on ACT, softmax-normalize + out-proj
evacuations + LN stats on DVE, broadcasts/adds/collectives on Pool (which
cannot touch PSUM on real HW). All bulk loads issue from the SP queue in
need order (an engine whose HWDGE ring fills blocks its own SEQ for >10us,
so ACT/Pool issue no early DMAs). Scheduler-only no_sync_barrier fences pin the LN
chains to the tail so their collective waits never head-of-line block the
pipeline.

Mask handling: tiles are classified on the host from the actual mask input:
fully-masked tiles are skipped, each computed tile gets a column window
[c_lo, 512) excluding fully-masked columns, and a narrow band [b0, b1) where
exp() output is multiplied by a 0/1 keep matrix. The causal structure is
exploited without being hardcoded. LayerNorm takes a fused path when
ln_g==1/ln_b==0 (checked on the host, part of the build cache key).
"""
import sys

if "/opt/trn_rl_repo" not in sys.path:
    sys.path.insert(0, "/opt/trn_rl_repo")

from contextlib import ExitStack

import numpy as np
import ml_dtypes

import concourse.bacc as bacc
import concourse.mybir as mybir
import concourse.tile as tile
from concourse.bass_utils import run_bass_kernel_spmd

BF16 = ml_dtypes.bfloat16
FP8 = ml_dtypes.float8_e4m3fn
F32 = mybir.dt.float32
BF = mybir.dt.bfloat16
F8 = mybir.dt.float8e4

B, S, E, H = 2, 2048, 1024, 16
Dh = E // H
LN_EPS = 1e-5
N_CORES = 8
HL = 4            # local heads per core
GROUPS = [[0, 1, 2, 3], [4, 5, 6, 7]]
NS = 4            # s blocks of 512 (query positions)
SBLK = 512
NT = 16           # t chunks of 128 (key positions)
TBLK = 128
NE = 8            # e chunks of 128 (contraction over E)
NEP = 4           # e chunk pairs (DoubleRow)
WSCALE = 16.0     # host-side weight scale (de-scaled at evacuation)
Act = mybir.ActivationFunctionType
Alu = mybir.AluOpType
DR = mybir.MatmulPerfMode.DoubleRow

_BUILD_CACHE = {}


def _dn128(x):
    return (x // 128) * 128


def _up128(x):
    return -(-x // 128) * 128


def classify_mask(mask):
    """Host-side tile classification from the actual mask input.

    Returns a structure tuple:
      tiles[sb] = list of (j, c_lo, band) with band = (b0, b1, keep_idx) or None
      kw = keep tensor width
    Shared across batches (union), so one SPMD program serves all cores.
    """
    tiles = []
    mult_list = []   # (sb, j, b0, b1)
    for sb in range(NS):
        lst = []
        for j in range(NT):
            # region[b] = mask[b, s-rows, t-cols]; col c of tile = s index
            reg = mask[:, sb * SBLK:(sb + 1) * SBLK, j * TBLK:(j + 1) * TBLK]
            keep_any_col = (~reg).any(axis=2).any(axis=0)     # [SBLK] per s col
            if not keep_any_col.any():
                continue  # fully masked in every batch
            c_lo = _dn128(int(np.argmax(keep_any_col)))
            # prefix [0, c_lo) must be fully masked in all batches (guaranteed
            # since keep_any_col is False there)
            masked_any_col = reg.any(axis=2).any(axis=0)      # [SBLK]
            masked_any_col[:c_lo] = False
            if masked_any_col.any():
                nz = np.nonzero(masked_any_col)[0]
                b0 = max(c_lo, _dn128(int(nz[0])))
                b1 = min(SBLK, _up128(int(nz[-1]) + 1))
                lst.append((j, c_lo, (b0, b1, len(mult_list))))
                mult_list.append((sb, j, b0, b1))
            else:
                lst.append((j, c_lo, None))
        # widest window first so the ctx accumulation group starts with a
        # full-bank clear
        lst.sort(key=lambda t: (t[1], t[0]))
        tiles.append(tuple(lst))
    kw = max([b1 - b0 for (_, _, b0, b1) in mult_list], default=128)
    return tuple(tiles), tuple(mult_list), kw


def build(tiles, mult_list, kw, trivial_gb=False):
    nm = max(1, len(mult_list))
    nc = bacc.Bacc("TRN2", num_devices=N_CORES)

    # ---- I/O -------------------------------------------------------------
    qT_d = nc.dram_tensor("qT", [E, S], F8, kind="ExternalInput")
    kT_d = nc.dram_tensor("kT", [E, S], F8, kind="ExternalInput")
    vT_d = nc.dram_tensor("vT", [E, S], F8, kind="ExternalInput")
    # weights pre-arranged to [128, NEP, 2, 256] (flattened) on the host
    wq_d = nc.dram_tensor("wq", [128, NEP * 2 * 256], F8, kind="ExternalInput")
    wk_d = nc.dram_tensor("wk", [128, NEP * 2 * 256], F8, kind="ExternalInput")
    wv_d = nc.dram_tensor("wv", [128, NEP * 2 * 256], F8, kind="ExternalInput")
    qb_d = nc.dram_tensor("qb", [128, 2], F32, kind="ExternalInput")
    kb_d = nc.dram_tensor("kb", [128, 2], F32, kind="ExternalInput")
    # keep is host-packed p-major ([128, nm*kw]) for 2KB DMA descriptors
    keep_d = nc.dram_tensor("keep", [TBLK, nm * kw], BF, kind="ExternalInput")
    wo_d = nc.dram_tensor("wo", [256, E], F8, kind="ExternalInput")
    resid_d = nc.dram_tensor("resid", [512, E], BF, kind="ExternalInput")
    lng_d = nc.dram_tensor("lng", [1, E], F32, kind="ExternalInput")
    lnb_d = nc.dram_tensor("lnb", [1, E], F32, kind="ExternalInput")
    out_d = nc.dram_tensor("out", [512, E], F32, kind="ExternalOutput")

    rs_in = nc.dram_tensor("rs_in", [S, E], BF, kind="Internal")
    rs_out = nc.dram_tensor("rs_out", [512, E], BF, kind="Internal")

    with tile.TileContext(nc) as tc, ExitStack() as ctx:
        # ---- persistent SBUF tiles --------------------------------------
        persist = ctx.enter_context(tc.tile_pool(name="persist", bufs=1))
        # x split into sb0 / sb1 / sb2+3 tiles so attention(0) can start as
        # soon as the first 1.5MB lands instead of after the full 6MB
        x_qs = [persist.tile([128, NE, SBLK], F8, name="x_q0"),
                persist.tile([128, NE, SBLK], F8, name="x_q1"),
                persist.tile([128, NE, 2 * SBLK], F8, name="x_q23")]
        x_ks = [persist.tile([128, NE, SBLK], F8, name="x_k0"),
                persist.tile([128, NE, SBLK], F8, name="x_k1"),
                persist.tile([128, NE, 2 * SBLK], F8, name="x_k23")]
        x_vs = [persist.tile([128, NE, SBLK], F8, name="x_v0"),
                persist.tile([128, NE, SBLK], F8, name="x_v1"),
                persist.tile([128, NE, 2 * SBLK], F8, name="x_v23")]
        q_all = persist.tile([128, 2, S], BF, name="q_all")   # [d-pair, dg, s]
        k_all = persist.tile([128, 2, S], BF, name="k_all")
        v_all = persist.tile([128, NT, HL, 65], BF, name="v_all")  # [t, j, h, d|1]
        att_sb = persist.tile([128, 2, S], F8, name="att_sb")  # [d-pair, dg, s]
        keep_sb = persist.tile([128, nm, kw], BF, name="keep_sb")
        qb_sb = persist.tile([128, 2], F32, name="qb_sb")
        kb_sb = persist.tile([128, 2], F32, name="kb_sb")
        wq_sb = persist.tile([128, NEP, 2, 256], F8, name="wq_sb")
        wk_sb = persist.tile([128, NEP, 2, 256], F8, name="wk_sb")
        wv_sb = persist.tile([128, NEP, 2, 256], F8, name="wv_sb")
        g_bc = persist.tile([128, E], F32, name="g_bc")
        b_bc = persist.tile([128, E], F32, name="b_bc")
        magic = persist.tile([128, 1], mybir.dt.uint32, name="magic")
        wo_sb = persist.tile([128, 2, E], F8, name="wo_sb")    # [d-pair, dg, e]
        resid_sb = persist.tile([128, NS, E], BF, name="resid_sb")

        nc.vector.memset(magic, 0x5F3759DF)
        # ones column for the row-sum trick
        nc.vector.memset(v_all[:, :, :, 64:65], 1.0)

        # ---- upfront DMA loads --------------------------------------------
        # ALL on the SP queue: ACT must issue no bulk DMAs (a full HWDGE ring
        # blocks its SEQ for >10us, delaying the first evacuations/exps) and
        # SP has no compute until the first rs_in DMAs ~30us in. The DMA
        # engine pool drains FIFO, so issue order == need order: each proj's
        # weight immediately before its x slice (k first: it heads the PE
        # stream), then keep (first band exps), x1, wo (outproj(0)), x23.
        # resid is loaded per-sblock inside proj_rs.
        XCOLS = [slice(0, SBLK), slice(SBLK, 2 * SBLK), slice(2 * SBLK, S)]
        nc.sync.dma_start(out=wk_sb,
                          in_=wk_d.rearrange("p (a b c) -> p a b c", a=NEP, b=2))
        nc.sync.dma_start(out=kb_sb, in_=kb_d[:, :])
        nc.sync.dma_start(
            out=x_ks[0], in_=kT_d[:, XCOLS[0]].rearrange("(c p) s -> p c s", p=128))
        nc.sync.dma_start(out=wq_sb,
                          in_=wq_d.rearrange("p (a b c) -> p a b c", a=NEP, b=2))
        nc.sync.dma_start(out=qb_sb, in_=qb_d[:, :])
        nc.sync.dma_start(
            out=x_qs[0], in_=qT_d[:, XCOLS[0]].rearrange("(c p) s -> p c s", p=128))
        nc.sync.dma_start(out=wv_sb,
                          in_=wv_d.rearrange("p (a b c) -> p a b c", a=NEP, b=2))
        nc.sync.dma_start(
            out=x_vs[0], in_=vT_d[:, XCOLS[0]].rearrange("(c p) s -> p c s", p=128))
        nc.sync.dma_start(out=keep_sb,
                          in_=keep_d.rearrange("p (m s) -> p m s", m=nm))
        for gi in (1, 2):
            nc.sync.dma_start(
                out=x_qs[gi],
                in_=qT_d[:, XCOLS[gi]].rearrange("(c p) s -> p c s", p=128))
            nc.sync.dma_start(
                out=x_ks[gi],
                in_=kT_d[:, XCOLS[gi]].rearrange("(c p) s -> p c s", p=128))
            nc.sync.dma_start(
                out=x_vs[gi],
                in_=vT_d[:, XCOLS[gi]].rearrange("(c p) s -> p c s", p=128))
            if gi == 1:
                nc.sync.dma_start(out=wo_sb,
                                  in_=wo_d.rearrange("(c p) d -> p c d", p=128))

        # ---- pools -------------------------------------------------------
        pp_pool = ctx.enter_context(tc.tile_pool(name="pp", bufs=2, space="PSUM"))
        sc_pool = ctx.enter_context(tc.tile_pool(name="sc", bufs=2, space="PSUM"))
        ctx_pool = ctx.enter_context(tc.tile_pool(name="ctxp", bufs=2, space="PSUM"))
        probs_pool = ctx.enter_context(tc.tile_pool(name="probs", bufs=6))
        small = ctx.enter_context(tc.tile_pool(name="small", bufs=4))
        work = ctx.enter_context(tc.tile_pool(name="work", bufs=4))
        lnp = ctx.enter_context(tc.tile_pool(name="lnp", bufs=2))

        INV_W = 1.0 / WSCALE

        def xgrp(sb):
            # (tile-group index, column offset within the group) for sblock
            return (sb, 0) if sb < 2 else (2, (sb - 2) * SBLK)

        def qk_proj(sb, xts, w_sb, bias_sb, dst):
            gi, off = xgrp(sb)
            xs = slice(off, off + SBLK)
            ss = slice(sb * SBLK, (sb + 1) * SBLK)
            for dg in range(2):
                ps = pp_pool.tile([128, SBLK], F32, name="ps", tag="pp")
                for ep in range(NEP):
                    nc.tensor.matmul(
                        ps,
                        w_sb[:, ep, :, dg * 128:(dg + 1) * 128],
                        xts[gi][:, 2 * ep:2 * ep + 2, xs],
                        start=(ep == 0), stop=(ep == NEP - 1),
                        perf_mode=DR,
                    )
                # evacuation on ACT: fused 1/WSCALE descale + bias add
                nc.scalar.activation(dst[:, dg, ss], ps, Act.Identity,
                                     bias=bias_sb[:, dg:dg + 1], scale=INV_W)

        def v_proj(sb):
            gi, off = xgrp(sb)
            # v[t, (h d)] for the 4 t-tiles of this sblock, two t-tiles per
            # PSUM bank
            for jj in range(2):
                j0 = 4 * sb + 2 * jj
                psv = pp_pool.tile([128, SBLK], F32, name="ps", tag="pp")
                for tl in range(2):
                    toff = off + (2 * jj + tl) * TBLK
                    ts = slice(toff, toff + TBLK)
                    for ep in range(NEP):
                        nc.tensor.matmul(
                            psv[:, tl * 256:(tl + 1) * 256],
                            x_vs[gi][:, 2 * ep:2 * ep + 2, ts],
                            wv_sb[:, ep, :, :],
                            start=(ep == 0), stop=(ep == NEP - 1),
                            perf_mode=DR,
                        )
                # v gets an extra x16 (cancelling 1/WSCALE exactly): att_sb
                # is carried at 16x so its fp8 quantization error is relative,
                # and the 1/(16*16) is folded into the outproj evacuation
                nc.scalar.activation(
                    v_all[:, j0:j0 + 2, :, 0:64],
                    psv.rearrange("p (j h d) -> p j h d", j=2, h=HL),
                    Act.Identity, scale=1.0)

        def attention(sb):
            tlist = tiles[sb]

            def normalize(dg, cA, cB):
                # recip from an SBUF copy of the PSUM ones-row (the custom-DVE
                # recip op reads garbage from PSUM on real HW), broadcast on
                # Pool; this chain gates outproj. For the LAST block (whose
                # chain is fully exposed in the tail) the sums copies run in
                # parallel on ACT (idle after the final exp) and DVE, and the
                # normalize muls split into column chunks so the outproj
                # chunks emitted right after can pipeline with them.
                last_blk = sb == NS - 1 and dg == 1
                for i, cx in ((0, cA), (1, cB)):
                    sums = small.tile([1, SBLK], F32, name="sums", tag="sums")
                    ceng = nc.scalar if (last_blk and i == 0) else nc.vector
                    if ceng is nc.scalar:
                        ceng.activation(sums, cx[64:65, :], Act.Identity)
                    else:
                        ceng.tensor_copy(sums, cx[64:65, :])
                    recip = small.tile([1, SBLK], F32, name="recip", tag="recip")
                    nc.vector.reciprocal_approx_fast(recip, sums)
                    bc = work.tile([64, SBLK], F32, name="bc", tag="bc")
                    nc.gpsimd.partition_broadcast(bc, recip, channels=64)
                    ncols = 4 if last_blk else 1
                    for cchunk in range(ncols):
                        colw = SBLK // ncols
                        c0 = cchunk * colw
                        nc.vector.tensor_mul(
                            att_sb[64 * i:64 * i + 64, dg,
                                   sb * SBLK + c0:sb * SBLK + c0 + colw],
                            cx[0:64, c0:c0 + colw], bc[:, c0:c0 + colw])

            # single flattened (dg, tile) stream with one tile of lookahead
            # ACROSS the dg boundary so ACT never drains mid-block
            cAB = {}
            pend = None

            def emit_ctx(p):
                _dg, _j, _cs, _p2, _first, _last = p
                cA, cB = cAB[_dg]
                nc.tensor.matmul(cA[:, _cs], v_all[:, _j, 2 * _dg, :],
                                 _p2[:, 0, _cs], start=_first, stop=_last)
                nc.tensor.matmul(cB[:, _cs], v_all[:, _j, 2 * _dg + 1, :],
                                 _p2[:, 1, _cs], start=_first, stop=_last)
                if _last:
                    normalize(_dg, cA, cB)

            for dg in range(2):
                for idx, (j, c_lo, band) in enumerate(tlist):
                    first, last = idx == 0, idx == len(tlist) - 1
                    if first:
                        cAB[dg] = (
                            ctx_pool.tile([65, SBLK], F32, name="cA", tag="ctx"),
                            ctx_pool.tile([65, SBLK], F32, name="cB", tag="ctx"))
                    # two-bank scores tile: head A in half 0, head B in half 1
                    sc2 = sc_pool.tile([128, 2, SBLK], F32, name="sc2", tag="sc")
                    ts = slice(j * TBLK, (j + 1) * TBLK)
                    ss = slice(sb * SBLK + c_lo, (sb + 1) * SBLK)
                    cs = slice(c_lo, SBLK)
                    nc.tensor.matmul(sc2[:, 0, cs], k_all[0:64, dg, ts],
                                     q_all[0:64, dg, ss], start=True, stop=True)
                    nc.tensor.matmul(sc2[:, 1, cs], k_all[64:128, dg, ts],
                                     q_all[64:128, dg, ss], start=True, stop=True)
                    p2 = probs_pool.tile([128, 2, SBLK], BF, name="p2", tag="pr")
                    nc.scalar.activation(p2[:, :, cs], sc2[:, :, cs], Act.Exp,
                                         scale=0.125)
                    if band is not None:
                        b0, b1, mi = band
                        bs_ = slice(b0, b1)
                        keep_b = keep_sb[:, mi:mi + 1, 0:b1 - b0].to_broadcast(
                            [128, 2, b1 - b0])
                        nc.vector.tensor_mul(p2[:, :, bs_], p2[:, :, bs_], keep_b)
                    if pend is not None:
                        emit_ctx(pend)
                    pend = (dg, j, cs, p2, first, last)
            emit_ctx(pend)

        def proj_rs(sb):
            # out projection partials + ReduceScatter for this sblock
            if sb == 1 and not trivial_gb:
                # LN consts, needed first at post_ln(0) (emitted two sblocks
                # later): issued here to stay clear of the x-chunk burst
                nc.gpsimd.dma_start(out=g_bc, in_=lng_d[0:1, :].to_broadcast([128, E]))
                nc.gpsimd.dma_start(out=b_bc, in_=lnb_d[0:1, :].to_broadcast([128, E]))
            # residual rows for this sblock's LN, prefetched well before use
            nc.sync.dma_start(out=resid_sb[:, sb, :],
                              in_=resid_d[sb * 128:(sb + 1) * 128, :])
            for sc in range(4):   # 128-row chunks within sblock
                srow = sb * 4 + sc
                row = slice(srow * 128, (srow + 1) * 128)
                pcopy = work.tile([128, E], BF, name="pcopy", tag="pcopy")
                for eo in range(2):
                    pp = pp_pool.tile([128, SBLK], F32, name="ps", tag="pp")
                    # both dg halves contract in ONE DoubleRow fp8 matmul
                    nc.tensor.matmul(
                        pp,
                        att_sb[:, :, row],
                        wo_sb[:, :, eo * SBLK:(eo + 1) * SBLK],
                        start=True, stop=True,
                        perf_mode=DR,
                    )
                    # 1/256 undoes the x16 on att_sb and the x16 on wo.
                    # For the last sblock ACT is past its final exp, so
                    # splitting the evacuations ACT/DVE halves the stagger
                    # on the rs_in(3) critical path.
                    if sb == NS - 1 and eo == 0:
                        nc.scalar.activation(pcopy[:, 0:SBLK], pp,
                                             Act.Identity, scale=1.0 / 256.0)
                    else:
                        nc.vector.tensor_scalar_mul(
                            pcopy[:, eo * SBLK:(eo + 1) * SBLK], pp, 1.0 / 256.0)
                nc.sync.dma_start(out=rs_in[row, :], in_=pcopy)
            nc.gpsimd.collective_compute(
                "ReduceScatter", Alu.add,
                ins=[rs_in[sb * SBLK:(sb + 1) * SBLK, :]],
                outs=[rs_out[sb * 128:(sb + 1) * 128, :]],
                replica_groups=GROUPS,
            )

        def post_ln(sb, act_rsqrt=False):
            # residual + LN on own 128 rows of this sblock (runs one sblock
            # behind the RS so its waits never head-of-line block the queues)
            pchunk = lnp.tile([128, E], BF, name="pchunk", tag="pchunk")
            x_t = lnp.tile([128, E], F32, name="x_t", tag="x_t")
            stats = small.tile([128, 2, 6], F32, name="stats", tag="stats")
            for h in range(2):
                hs = slice(h * 512, (h + 1) * 512)
                deng = nc.sync if h == 0 else nc.scalar
                deng.dma_start(out=pchunk[:, hs],
                               in_=rs_out[sb * 128:(sb + 1) * 128, hs])
                eng = nc.gpsimd if h == 0 else nc.vector
                eng.tensor_add(x_t[:, hs], resid_sb[:, sb, hs], pchunk[:, hs])
                nc.vector.bn_stats(stats[:, h, :], x_t[:, hs])
            mv = small.tile([128, 2], F32, name="mv", tag="mv")
            nc.vector.bn_aggr(mv, stats)
            # rstd = rsqrt(var + eps) on DVE (bit-trick seed + 2 Newton iters)
            # so ACT never leaves the exp table set
            U32 = mybir.dt.uint32
            ws = small.tile([128, 1], F32, name="ws", tag="ws")
            nc.vector.tensor_scalar_add(ws, mv[:, 1:2], LN_EPS)
            rstd = small.tile([128, 1], F32, name="rstd", tag="rstd")
            if act_rsqrt:
                # tail LNs run after the last exp: the ACT table switch is
                # free to take there; Sqrt+reciprocal replaces the Newton
                # chain (Rsqrt itself is blocked for accuracy)
                sq = small.tile([128, 1], F32, name="sq", tag="sq")
                nc.scalar.activation(sq, ws, Act.Sqrt)
                nc.vector.reciprocal(rstd, sq)
            else:
                hbits = small.tile([128, 1], U32, name="hbits", tag="hbits")
                nc.vector.tensor_scalar(hbits, ws.bitcast(U32), 1, None,
                                        op0=Alu.logical_shift_right)
                nc.vector.scalar_tensor_tensor(
                    rstd.bitcast(U32), magic, 0, hbits, op0=Alu.bypass,
                    op1=Alu.subtract)
                nt = small.tile([128, 1], F32, name="nt", tag="nt")
                for _ in range(2):
                    nc.vector.tensor_mul(nt, ws, rstd)
                    nc.vector.tensor_mul(nt, nt, rstd)
                    nc.vector.tensor_scalar(nt, nt, -0.5, 1.5, op0=Alu.mult,
                                            op1=Alu.add)
                    nc.vector.tensor_mul(rstd, rstd, nt)
            o_t = lnp.tile([128, E], F32, name="o_t", tag="o_t")
            if trivial_gb:
                # ln_g==1, ln_b==0 (checked on the host): a single
                # (x-mu)*rstd per half, with the output DMA overlapping the
                # second half's compute
                for h in range(2):
                    hs = slice(h * 512, (h + 1) * 512)
                    nc.vector.tensor_scalar(o_t[:, hs], x_t[:, hs],
                                            mv[:, 0:1], rstd,
                                            op0=Alu.subtract, op1=Alu.mult)
                    eng = nc.sync if h == 0 else nc.scalar
                    eng.dma_start(out=out_d[sb * 128:(sb + 1) * 128, hs],
                                  in_=o_t[:, hs])
            else:
                y_t = lnp.tile([128, E], F32, name="y_t", tag="y_t")
                nc.vector.scalar_tensor_tensor(
                    y_t, x_t, mv[:, 0:1], g_bc, op0=Alu.subtract, op1=Alu.mult)
                nc.vector.scalar_tensor_tensor(
                    o_t, y_t, rstd, b_bc, op0=Alu.mult, op1=Alu.add)
                nc.sync.dma_start(out=out_d[sb * 128:(sb + 1) * 128, :], in_=o_t)

        # Pipeline: proj(sb+1) is emitted between attention(sb) and
        # outproj(sb) so the PE has work while the softmax-normalize chain
        # (recip -> broadcast -> mul) completes, instead of idling into a
        # low p-state.
        def kv_proj(sb):
            qk_proj(sb, x_ks, wk_sb, kb_sb, k_all)
            v_proj(sb)

        # Attention visit order [1, 2, 3, 0]: the ReduceScatter chain starts
        # after the second-smallest block (~32us) and stays continuous —
        # each rs_in arrives just as the previous collective finishes — and
        # compute ends on the smallest block, so only one RS+LN is exposed
        # in the tail. k/v projections still build incrementally (attention
        # (sb) needs k/v of every block up to sb).
        # proj_rs(sb) is deferred one block (emitted after attention(sb+1))
        # so the softmax-normalize chain of block sb never stalls the PE:
        # attention(sb+1) fills that window.
        kv_proj(0)
        qk_proj(0, x_qs, wq_sb, qb_sb, q_all)
        for sb in range(NS):
            attention(sb)
            if sb >= 1:
                proj_rs(sb - 1)
            if sb + 1 < NS:
                kv_proj(sb + 1)
                qk_proj(sb + 1, x_qs, wq_sb, qb_sb, q_all)
            if sb == NS - 1:
                proj_rs(sb)
            if sb == 2:
                # scheduler-only fences: without them the Tile scheduler
                # hoists the LN chains (whose first op waits on a
                # ReduceScatter) into the middle of the pipeline,
                # head-of-line blocking DVE/SP behind collective waits.
                tc.no_sync_barrier()
                post_ln(0)
        for sb in range(1, NS):
            # one fence per LN block: the scheduler otherwise reorders the
            # rs_out->pchunk DMAs across blocks, head-of-line blocking an
            # already-satisfied LN behind the last collective
            tc.no_sync_barrier()
            post_ln(sb, act_rsqrt=True)

    nc.finalize()
    return nc


def _prep_core(inputs, b, g):
    heads = slice(HL * g, HL * (g + 1))
    query = np.asarray(inputs["query"][b], np.float32)
    key = np.asarray(inputs["key"][b], np.float32)
    value = np.asarray(inputs["value"][b], np.float32)
    Wq_w = np.asarray(inputs["Wq_w"], np.float32)
    Wk_w = np.asarray(inputs["Wk_w"], np.float32)
    Wv_w = np.asarray(inputs["Wv_w"], np.float32)
    Wq_b = np.asarray(inputs["Wq_b"], np.float32)
    Wk_b = np.asarray(inputs["Wk_b"], np.float32)
    Wv_b = np.asarray(inputs["Wv_b"], np.float32)
    out_w = np.asarray(inputs["out_w"], np.float32)
    out_b = np.asarray(inputs["out_b"], np.float32)

    def packb(t):  # [4, 64] -> [128, 2] pair-major
        return np.ascontiguousarray(
            t.reshape(2, 2, Dh).transpose(1, 2, 0).reshape(128, 2))

    def packw(Wh):  # [4, Dh, E] head-major -> [128, NEP*2*256] DR layout
        w = Wh.reshape(256, E).T * WSCALE            # [E, 256]
        w = w.reshape(NEP, 2, 128, 256).transpose(2, 0, 1, 3)
        return np.ascontiguousarray(w.reshape(128, NEP * 2 * 256)).astype(FP8)

    d = {
        "qT": np.ascontiguousarray(query.T).astype(FP8),
        "kT": np.ascontiguousarray(key.T).astype(FP8),
        "vT": np.ascontiguousarray(value.T).astype(FP8),
        "wq": packw(Wq_w[heads]),
        "wk": packw(Wk_w[heads]),
        "wv": packw(Wv_w[heads]),
        "qb": packb(Wq_b[heads]),
        "kb": packb(Wk_b[heads]),
        "wo": np.ascontiguousarray(
            out_w[:, 256 * g:256 * (g + 1)].T * WSCALE).astype(FP8),
        "lng": np.asarray(inputs["ln_g"], np.float32).reshape(1, E).copy(),
        "lnb": np.asarray(inputs["ln_b"], np.float32).reshape(1, E).copy(),
    }
    const = out_b + Wv_b.reshape(E) @ out_w.T
    rows = query.reshape(NS, 4, 128, E)[:, g, :, :].reshape(512, E)
    d["resid"] = np.ascontiguousarray(rows + const[None, :]).astype(BF16)
    return d


def _prep_keep(mask, mult_list, kw, b):
    """Keep matrix packed p-major: [TBLK, nm*kw] bf16."""
    nm = max(1, len(mult_list))
    keep = np.zeros((TBLK, nm, kw), np.float32)
    for mi, (sb, j, b0, b1) in enumerate(mult_list):
        reg = mask[b, sb * SBLK + b0:sb * SBLK + b1,
                   j * TBLK:(j + 1) * TBLK]
        keep[:, mi, 0:b1 - b0] = (~reg).T.astype(np.float32)
    return np.ascontiguousarray(keep.reshape(TBLK, nm * kw)).astype(BF16)


def kernel(**inputs):
    mask = np.asarray(inputs["mask"], bool)
    tiles, mult_list, kw = classify_mask(mask)
    trivial_gb = bool(np.all(np.asarray(inputs["ln_g"]) == 1.0)
                      and np.all(np.asarray(inputs["ln_b"]) == 0.0))
    key_struct = (tiles, mult_list, kw, trivial_gb)
    if key_struct not in _BUILD_CACHE:
        _BUILD_CACHE[key_struct] = build(tiles, mult_list, kw, trivial_gb)
    nc = _BUILD_CACHE[key_struct]

    in_maps = []
    for c in range(N_CORES):
        b, g = c // 4, c % 4
        d = _prep_core(inputs, b, g)
        d["keep"] = _prep_keep(mask, mult_list, kw, b)
        in_maps.append(d)

    res = run_bass_kernel_spmd(nc, in_maps, core_ids=list(range(N_CORES)))

    out = np.empty((B, S, E), np.float32)
    for c in range(N_CORES):
        b, g = c // 4, c % 4
        o = res.results[c]["out"]  # [512, E]
        for sb in range(NS):
            out[b, sb * SBLK + 128 * g: sb * SBLK + 128 * (g + 1), :] = \
                o[sb * 128:(sb + 1) * 128, :]
    return out


# revision 69
# speedup vs baseline: 1.3038x; 1.2947x over previous
"""Distributed MultiHeadAttention + residual + LayerNorm for 8 TRN2 NeuronCores.

Problem: B=2, S=2048, E=1024, H=16, Dh=64 (fp32 in/out).
Sharding: core c = (batch b=c//4, head-group g=c%4); each core computes 4 heads
for its batch. Output projection is row-sharded on the att dim; partials are
combined with per-sblock 4-rank ReduceScatter, then residual+LayerNorm happen
on each core's own row shard.

Single fused pipeline per 512-row sblock:
  [qkv-proj(sb) -> attention(sb) -> (deferred) out-proj(sb-1) -> RS(sb-1)]
The out-projection of block sb is emitted after attention(sb+1) so its
softmax-normalize chain (recip -> partition_broadcast -> mul) never idles the
PE into a low p-state; the ReduceScatter chain runs continuously behind
compute and only the last block's RS + LayerNorm are exposed in the tail.

fp8e4(DoubleRow) matmuls (2 k-tiles per pass = 2x PE throughput) are used for
the QKV projections (x and weights both e4m3; weights pre-scaled by 16 on the
host to stay clear of e4m3 subnormals, de-scaled at PSUM evacuation) and for
the output projection (att and wo both carried at 16x in fp8; both dg halves
contract in one DoubleRow matmul; the 1/256 folds into the evacuation).

Scores are computed transposed (scoresT[t, s]) in bf16 so softmax row sums
come out of the ctx matmul for free via a ones-column appended to V; the
1/sqrt(Dh) scale folds into the exp activation (scale=0.125). The flattened
(dg, tile) loop runs with one tile of lookahead (scores(t+1) before ctx(t))
so PE and the exp-saturated ACT engine pipeline instead of ping-ponging.
Engine placement: exp + proj evacuations on ACT, softmax-normalize + out-proj
evacuations + LN stats on DVE, broadcasts/adds/collectives on Pool (which
cannot touch PSUM on real HW). All bulk loads issue from the SP queue in
need order (an engine whose HWDGE ring fills blocks its own SEQ for >10us,
so ACT/Pool issue no early DMAs). Scheduler-only no_sync_barrier fences pin the LN
chains to the tail so their collective waits never head-of-line block the
pipeline.

Mask handling: tiles are classified on the host from the actual mask input:
fully-masked tiles are skipped, each computed tile gets a column window
[c_lo, 512) excluding fully-masked columns, and a narrow band [b0, b1) where
exp() output is multiplied by a 0/1 keep matrix. The causal structure is
exploited without being hardcoded. LayerNorm takes a fused path when
ln_g==1/ln_b==0 (checked on the host, part of the build cache key).
"""
import sys

if "/opt/trn_rl_repo" not in sys.path:
    sys.path.insert(0, "/opt/trn_rl_repo")

from contextlib import ExitStack

import numpy as np
import ml_dtypes

import concourse.bacc as bacc
import concourse.mybir as mybir
import concourse.tile as tile
from concourse.bass_utils import run_bass_kernel_spmd

BF16 = ml_dtypes.bfloat16
FP8 = ml_dtypes.float8_e4m3fn
F32 = mybir.dt.float32
BF = mybir.dt.bfloat16
F8 = mybir.dt.float8e4

B, S, E, H = 2, 2048, 1024, 16
Dh = E // H
LN_EPS = 1e-5
N_CORES = 8
HL = 4            # local heads per core
GROUPS = [[0, 1, 2, 3], [4, 5, 6, 7]]
NS = 4            # s blocks of 512 (query positions)
SBLK = 512
NT = 16           # t chunks of 128 (key positions)
TBLK = 128
NE = 8            # e chunks of 128 (contraction over E)
NEP = 4           # e chunk pairs (DoubleRow)
WSCALE = 16.0     # host-side weight scale (de-scaled at evacuation)
Act = mybir.ActivationFunctionType
Alu = mybir.AluOpType
DR = mybir.MatmulPerfMode.DoubleRow

_BUILD_CACHE = {}


def _dn128(x):
    return (x // 128) * 128


def _up128(x):
    return -(-x // 128) * 128


def classify_mask(mask):
    """Host-side tile classification from the actual mask input.

    Returns a structure tuple:
      tiles[sb] = list of (j, c_lo, band) with band = (b0, b1, keep_idx) or None
      kw = keep tensor width
    Shared across batches (union), so one SPMD program serves all cores.
    """
    tiles = []
    mult_list = []   # (sb, j, b0, b1)
    for sb in range(NS):
        lst = []
        for j in range(NT):
            # region[b] = mask[b, s-rows, t-cols]; col c of tile = s index
            reg = mask[:, sb * SBLK:(sb + 1) * SBLK, j * TBLK:(j + 1) * TBLK]
            keep_any_col = (~reg).any(axis=2).any(axis=0)     # [SBLK] per s col
            if not keep_any_col.any():
                continue  # fully masked in every batch
            c_lo = _dn128(int(np.argmax(keep_any_col)))
            # prefix [0, c_lo) must be fully masked in all batches (guaranteed
            # since keep_any_col is False there)
            masked_any_col = reg.any(axis=2).any(axis=0)      # [SBLK]
            masked_any_col[:c_lo] = False
            if masked_any_col.any():
                nz = np.nonzero(masked_any_col)[0]
                b0 = max(c_lo, _dn128(int(nz[0])))
                b1 = min(SBLK, _up128(int(nz[-1]) + 1))
                lst.append((j, c_lo, (b0, b1, len(mult_list))))
                mult_list.append((sb, j, b0, b1))
            else:
                lst.append((j, c_lo, None))
        # widest window first so the ctx accumulation group starts with a
        # full-bank clear
        lst.sort(key=lambda t: (t[1], t[0]))
        tiles.append(tuple(lst))
    kw = max([b1 - b0 for (_, _, b0, b1) in mult_list], default=128)
    return tuple(tiles), tuple(mult_list), kw


def build(tiles, mult_list, kw, trivial_gb=False):
    nm = max(1, len(mult_list))
    nc = bacc.Bacc("TRN2", num_devices=N_CORES)

    # ---- I/O -------------------------------------------------------------
    qT_d = nc.dram_tensor("qT", [E, S], F8, kind="ExternalInput")
    kT_d = nc.dram_tensor("kT", [E, S], F8, kind="ExternalInput")
    vT_d = nc.dram_tensor("vT", [E, S], F8, kind="ExternalInput")
    # weights pre-arranged to [128, NEP, 2, 256] (flattened) on the host
    wq_d = nc.dram_tensor("wq", [128, NEP * 2 * 256], F8, kind="ExternalInput")
    wk_d = nc.dram_tensor("wk", [128, NEP * 2 * 256], F8, kind="ExternalInput")
    wv_d = nc.dram_tensor("wv", [128, NEP * 2 * 256], F8, kind="ExternalInput")
    qb_d = nc.dram_tensor("qb", [128, 2], F32, kind="ExternalInput")
    kb_d = nc.dram_tensor("kb", [128, 2], F32, kind="ExternalInput")
    # keep is host-packed p-major ([128, nm*kw]) for 2KB DMA descriptors
    keep_d = nc.dram_tensor("keep", [TBLK, nm * kw], BF, kind="ExternalInput")
    wo_d = nc.dram_tensor("wo", [256, E], F8, kind="ExternalInput")
    resid_d = nc.dram_tensor("resid", [512, E], BF, kind="ExternalInput")
    lng_d = nc.dram_tensor("lng", [1, E], F32, kind="ExternalInput")
    lnb_d = nc.dram_tensor("lnb", [1, E], F32, kind="ExternalInput")
    out_d = nc.dram_tensor("out", [512, E], F32, kind="ExternalOutput")

    rs_in = nc.dram_tensor("rs_in", [S, E], BF, kind="Internal")
    rs_out = nc.dram_tensor("rs_out", [512, E], BF, kind="Internal")

    with tile.TileContext(nc) as tc, ExitStack() as ctx:
        # ---- persistent SBUF tiles --------------------------------------
        persist = ctx.enter_context(tc.tile_pool(name="persist", bufs=1))
        # x split into sb0 / sb1 / sb2+3 tiles so attention(0) can start as
        # soon as the first 1.5MB lands instead of after the full 6MB
        x_qs = [persist.tile([128, NE, SBLK], F8, name="x_q0"),
                persist.tile([128, NE, SBLK], F8, name="x_q1"),
                persist.tile([128, NE, 2 * SBLK], F8, name="x_q23")]
        x_ks = [persist.tile([128, NE, SBLK], F8, name="x_k0"),
                persist.tile([128, NE, SBLK], F8, name="x_k1"),
                persist.tile([128, NE, 2 * SBLK], F8, name="x_k23")]
        x_vs = [persist.tile([128, NE, SBLK], F8, name="x_v0"),
                persist.tile([128, NE, SBLK], F8, name="x_v1"),
                persist.tile([128, NE, 2 * SBLK], F8, name="x_v23")]
        q_all = persist.tile([128, 2, S], BF, name="q_all")   # [d-pair, dg, s]
        k_all = persist.tile([128, 2, S], BF, name="k_all")
        v_all = persist.tile([128, NT, HL, 65], BF, name="v_all")  # [t, j, h, d|1]
        att_sb = persist.tile([128, 2, S], F8, name="att_sb")  # [d-pair, dg, s]
        keep_sb = persist.tile([128, nm, kw], BF, name="keep_sb")
        qb_sb = persist.tile([128, 2], F32, name="qb_sb")
        kb_sb = persist.tile([128, 2], F32, name="kb_sb")
        wq_sb = persist.tile([128, NEP, 2, 256], F8, name="wq_sb")
        wk_sb = persist.tile([128, NEP, 2, 256], F8, name="wk_sb")
        wv_sb = persist.tile([128, NEP, 2, 256], F8, name="wv_sb")
        g_bc = persist.tile([128, E], F32, name="g_bc")
        b_bc = persist.tile([128, E], F32, name="b_bc")
        magic = persist.tile([128, 1], mybir.dt.uint32, name="magic")
        wo_sb = persist.tile([128, 2, E], F8, name="wo_sb")    # [d-pair, dg, e]
        resid_sb = persist.tile([128, NS, E], BF, name="resid_sb")

        nc.vector.memset(magic, 0x5F3759DF)
        # ones column for the row-sum trick
        nc.vector.memset(v_all[:, :, :, 64:65], 1.0)

        # ---- upfront DMA loads --------------------------------------------
        # ALL on the SP queue: ACT must issue no bulk DMAs (a full HWDGE ring
        # blocks its SEQ for >10us, delaying the first evacuations/exps) and
        # SP has no compute until the first rs_in DMAs ~30us in. The DMA
        # engine pool drains FIFO, so issue order == need order: each proj's
        # weight immediately before its x slice (k first: it heads the PE
        # stream), then keep (first band exps), x1, wo (outproj(0)), x23.
        # resid is loaded per-sblock inside proj_rs.
        XCOLS = [slice(0, SBLK), slice(SBLK, 2 * SBLK), slice(2 * SBLK, S)]
        nc.sync.dma_start(out=wk_sb,
                          in_=wk_d.rearrange("p (a b c) -> p a b c", a=NEP, b=2))
        nc.sync.dma_start(out=kb_sb, in_=kb_d[:, :])
        nc.sync.dma_start(
            out=x_ks[0], in_=kT_d[:, XCOLS[0]].rearrange("(c p) s -> p c s", p=128))
        nc.sync.dma_start(out=wq_sb,
                          in_=wq_d.rearrange("p (a b c) -> p a b c", a=NEP, b=2))
        nc.sync.dma_start(out=qb_sb, in_=qb_d[:, :])
        nc.sync.dma_start(
            out=x_qs[0], in_=qT_d[:, XCOLS[0]].rearrange("(c p) s -> p c s", p=128))
        nc.sync.dma_start(out=wv_sb,
                          in_=wv_d.rearrange("p (a b c) -> p a b c", a=NEP, b=2))
        nc.sync.dma_start(
            out=x_vs[0], in_=vT_d[:, XCOLS[0]].rearrange("(c p) s -> p c s", p=128))
        nc.sync.dma_start(out=keep_sb,
                          in_=keep_d.rearrange("p (m s) -> p m s", m=nm))
        for gi in (1, 2):
            nc.sync.dma_start(
                out=x_qs[gi],
                in_=qT_d[:, XCOLS[gi]].rearrange("(c p) s -> p c s", p=128))
            nc.sync.dma_start(
                out=x_ks[gi],
                in_=kT_d[:, XCOLS[gi]].rearrange("(c p) s -> p c s", p=128))
            nc.sync.dma_start(
                out=x_vs[gi],
                in_=vT_d[:, XCOLS[gi]].rearrange("(c p) s -> p c s", p=128))
            if gi == 1:
                nc.sync.dma_start(out=wo_sb,
                                  in_=wo_d.rearrange("(c p) d -> p c d", p=128))

        # ---- pools -------------------------------------------------------
        pp_pool = ctx.enter_context(tc.tile_pool(name="pp", bufs=2, space="PSUM"))
        sc_pool = ctx.enter_context(tc.tile_pool(name="sc", bufs=2, space="PSUM"))
        ctx_pool = ctx.enter_context(tc.tile_pool(name="ctxp", bufs=2, space="PSUM"))
        probs_pool = ctx.enter_context(tc.tile_pool(name="probs", bufs=6))
        small = ctx.enter_context(tc.tile_pool(name="small", bufs=4))
        work = ctx.enter_context(tc.tile_pool(name="work", bufs=4))
        lnp = ctx.enter_context(tc.tile_pool(name="lnp", bufs=2))

        INV_W = 1.0 / WSCALE

        def xgrp(sb):
            # (tile-group index, column offset within the group) for sblock
            return (sb, 0) if sb < 2 else (2, (sb - 2) * SBLK)

        def qk_proj(sb, xts, w_sb, bias_sb, dst):
            gi, off = xgrp(sb)
            xs = slice(off, off + SBLK)
            ss = slice(sb * SBLK, (sb + 1) * SBLK)
            for dg in range(2):
                ps = pp_pool.tile([128, SBLK], F32, name="ps", tag="pp")
                for ep in range(NEP):
                    nc.tensor.matmul(
                        ps,
                        w_sb[:, ep, :, dg * 128:(dg + 1) * 128],
                        xts[gi][:, 2 * ep:2 * ep + 2, xs],
                        start=(ep == 0), stop=(ep == NEP - 1),
                        perf_mode=DR,
                    )
                # evacuation on ACT: fused 1/WSCALE descale + bias add
                nc.scalar.activation(dst[:, dg, ss], ps, Act.Identity,
                                     bias=bias_sb[:, dg:dg + 1], scale=INV_W)

        def v_proj(sb):
            gi, off = xgrp(sb)
            # v[t, (h d)] for the 4 t-tiles of this sblock, two t-tiles per
            # PSUM bank
            for jj in range(2):
                j0 = 4 * sb + 2 * jj
                psv = pp_pool.tile([128, SBLK], F32, name="ps", tag="pp")
                for tl in range(2):
                    toff = off + (2 * jj + tl) * TBLK
                    ts = slice(toff, toff + TBLK)
                    for ep in range(NEP):
                        nc.tensor.matmul(
                            psv[:, tl * 256:(tl + 1) * 256],
                            x_vs[gi][:, 2 * ep:2 * ep + 2, ts],
                            wv_sb[:, ep, :, :],
                            start=(ep == 0), stop=(ep == NEP - 1),
                            perf_mode=DR,
                        )
                # v gets an extra x16 (cancelling 1/WSCALE exactly): att_sb
                # is carried at 16x so its fp8 quantization error is relative,
                # and the 1/(16*16) is folded into the outproj evacuation
                nc.scalar.activation(
                    v_all[:, j0:j0 + 2, :, 0:64],
                    psv.rearrange("p (j h d) -> p j h d", j=2, h=HL),
                    Act.Identity, scale=1.0)

        def attention(sb):
            tlist = tiles[sb]

            def normalize(dg, cA, cB):
                # recip from an SBUF copy of the PSUM ones-row (the custom-DVE
                # recip op reads garbage from PSUM on real HW), broadcast on
                # Pool; this chain gates outproj. For the LAST block (whose
                # chain is fully exposed in the tail) the sums copies run in
                # parallel on ACT (idle after the final exp) and DVE, and the
                # normalize muls split into column chunks so the outproj
                # chunks emitted right after can pipeline with them.
                last_blk = sb == NS - 1 and dg == 1
                for i, cx in ((0, cA), (1, cB)):
                    sums = small.tile([1, SBLK], F32, name="sums", tag="sums")
                    ceng = nc.scalar if (last_blk and i == 0) else nc.vector
                    if ceng is nc.scalar:
                        ceng.activation(sums, cx[64:65, :], Act.Identity)
                    else:
                        ceng.tensor_copy(sums, cx[64:65, :])
                    recip = small.tile([1, SBLK], F32, name="recip", tag="recip")
                    nc.vector.reciprocal_approx_fast(recip, sums)
                    bc = work.tile([64, SBLK], F32, name="bc", tag="bc")
                    nc.gpsimd.partition_broadcast(bc, recip, channels=64)
                    ncols = 4 if last_blk else 1
                    for cchunk in range(ncols):
                        colw = SBLK // ncols
                        c0 = cchunk * colw
                        nc.vector.tensor_mul(
                            att_sb[64 * i:64 * i + 64, dg,
                                   sb * SBLK + c0:sb * SBLK + c0 + colw],
                            cx[0:64, c0:c0 + colw], bc[:, c0:c0 + colw])

            # single flattened (dg, tile) stream with one tile of lookahead
            # ACROSS the dg boundary so ACT never drains mid-block
            cAB = {}
            pend = None

            def emit_ctx(p):
                _dg, _j, _cs, _p2, _first, _last = p
                cA, cB = cAB[_dg]
                nc.tensor.matmul(cA[:, _cs], v_all[:, _j, 2 * _dg, :],
                                 _p2[:, 0, _cs], start=_first, stop=_last)
                nc.tensor.matmul(cB[:, _cs], v_all[:, _j, 2 * _dg + 1, :],
                                 _p2[:, 1, _cs], start=_first, stop=_last)
                if _last:
                    normalize(_dg, cA, cB)

            for dg in range(2):
                for idx, (j, c_lo, band) in enumerate(tlist):
                    first, last = idx == 0, idx == len(tlist) - 1
                    if first:
                        cAB[dg] = (
                            ctx_pool.tile([65, SBLK], F32, name="cA", tag="ctx"),
                            ctx_pool.tile([65, SBLK], F32, name="cB", tag="ctx"))
                    # two-bank scores tile: head A in half 0, head B in half 1
                    sc2 = sc_pool.tile([128, 2, SBLK], F32, name="sc2", tag="sc")
                    ts = slice(j * TBLK, (j + 1) * TBLK)
                    ss = slice(sb * SBLK + c_lo, (sb + 1) * SBLK)
                    cs = slice(c_lo, SBLK)
                    nc.tensor.matmul(sc2[:, 0, cs], k_all[0:64, dg, ts],
                                     q_all[0:64, dg, ss], start=True, stop=True)
                    nc.tensor.matmul(sc2[:, 1, cs], k_all[64:128, dg, ts],
                                     q_all[64:128, dg, ss], start=True, stop=True)
                    p2 = probs_pool.tile([128, 2, SBLK], BF, name="p2", tag="pr")
                    nc.scalar.activation(p2[:, :, cs], sc2[:, :, cs], Act.Exp,
                                         scale=0.125)
                    if band is not None:
                        b0, b1, mi = band
                        bs_ = slice(b0, b1)
                        keep_b = keep_sb[:, mi:mi + 1, 0:b1 - b0].to_broadcast(
                            [128, 2, b1 - b0])
                        nc.vector.tensor_mul(p2[:, :, bs_], p2[:, :, bs_], keep_b)
                    if pend is not None:
                        emit_ctx(pend)
                    pend = (dg, j, cs, p2, first, last)
            emit_ctx(pend)

        def proj_rs(sb):
            # out projection partials + ReduceScatter for this sblock
            if sb == 1 and not trivial_gb:
                # LN consts, needed first at post_ln(0) (emitted two sblocks
                # later): issued here to stay clear of the x-chunk burst
                nc.gpsimd.dma_start(out=g_bc, in_=lng_d[0:1, :].to_broadcast([128, E]))
                nc.gpsimd.dma_start(out=b_bc, in_=lnb_d[0:1, :].to_broadcast([128, E]))
            # residual rows for this sblock's LN, prefetched well before use
            nc.sync.dma_start(out=resid_sb[:, sb, :],
                              in_=resid_d[sb * 128:(sb + 1) * 128, :])
            for sc in range(4):   # 128-row chunks within sblock
                srow = sb * 4 + sc
                row = slice(srow * 128, (srow + 1) * 128)
                pcopy = work.tile([128, E], BF, name="pcopy", tag="pcopy")
                for eo in range(2):
                    pp = pp_pool.tile([128, SBLK], F32, name="ps", tag="pp")
                    # both dg halves contract in ONE DoubleRow fp8 matmul
                    nc.tensor.matmul(
                        pp,
                        att_sb[:, :, row],
                        wo_sb[:, :, eo * SBLK:(eo + 1) * SBLK],
                        start=True, stop=True,
                        perf_mode=DR,
                    )
                    # 1/256 undoes the x16 on att_sb and the x16 on wo.
                    # For the last sblock ACT is past its final exp, so
                    # splitting the evacuations ACT/DVE halves the stagger
                    # on the rs_in(3) critical path.
                    if sb == NS - 1 and eo == 0:
                        nc.scalar.activation(pcopy[:, 0:SBLK], pp,
                                             Act.Identity, scale=1.0 / 256.0)
                    else:
                        nc.vector.tensor_scalar_mul(
                            pcopy[:, eo * SBLK:(eo + 1) * SBLK], pp, 1.0 / 256.0)
                nc.sync.dma_start(out=rs_in[row, :], in_=pcopy)
            nc.gpsimd.collective_compute(
                "ReduceScatter", Alu.add,
                ins=[rs_in[sb * SBLK:(sb + 1) * SBLK, :]],
                outs=[rs_out[sb * 128:(sb + 1) * 128, :]],
                replica_groups=GROUPS,
            )

        def post_ln(sb, act_rsqrt=False):
            # residual + LN on own 128 rows of this sblock (runs one sblock
            # behind the RS so its waits never head-of-line block the queues)
            pchunk = lnp.tile([128, E], BF, name="pchunk", tag="pchunk")
            x_t = lnp.tile([128, E], F32, name="x_t", tag="x_t")
            stats = small.tile([128, 2, 6], F32, name="stats", tag="stats")
            for h in range(2):
                hs = slice(h * 512, (h + 1) * 512)
                deng = nc.sync if h == 0 else nc.scalar
                deng.dma_start(out=pchunk[:, hs],
                               in_=rs_out[sb * 128:(sb + 1) * 128, hs])
                eng = nc.gpsimd if h == 0 else nc.vector
                eng.tensor_add(x_t[:, hs], resid_sb[:, sb, hs], pchunk[:, hs])
                nc.vector.bn_stats(stats[:, h, :], x_t[:, hs])
            mv = small.tile([128, 2], F32, name="mv", tag="mv")
            nc.vector.bn_aggr(mv, stats)
            # rstd = rsqrt(var + eps) on DVE (bit-trick seed + 2 Newton iters)
            # so ACT never leaves the exp table set
            U32 = mybir.dt.uint32
            ws = small.tile([128, 1], F32, name="ws", tag="ws")
            nc.vector.tensor_scalar_add(ws, mv[:, 1:2], LN_EPS)
            rstd = small.tile([128, 1], F32, name="rstd", tag="rstd")
            if act_rsqrt:
                # tail LNs run after the last exp: the ACT table switch is
                # free to take there; Sqrt+reciprocal replaces the Newton
                # chain (Rsqrt itself is blocked for accuracy)
                sq = small.tile([128, 1], F32, name="sq", tag="sq")
                nc.scalar.activation(sq, ws, Act.Sqrt)
                nc.vector.reciprocal(rstd, sq)
            else:
                hbits = small.tile([128, 1], U32, name="hbits", tag="hbits")
                nc.vector.tensor_scalar(hbits, ws.bitcast(U32), 1, None,
                                        op0=Alu.logical_shift_right)
                nc.vector.scalar_tensor_tensor(
                    rstd.bitcast(U32), magic, 0, hbits, op0=Alu.bypass,
                    op1=Alu.subtract)
                nt = small.tile([128, 1], F32, name="nt", tag="nt")
                for _ in range(2):
                    nc.vector.tensor_mul(nt, ws, rstd)
                    nc.vector.tensor_mul(nt, nt, rstd)
                    nc.vector.tensor_scalar(nt, nt, -0.5, 1.5, op0=Alu.mult,
                                            op1=Alu.add)
                    nc.vector.tensor_mul(rstd, rstd, nt)
            o_t = lnp.tile([128, E], F32, name="o_t", tag="o_t")
            if trivial_gb:
                # ln_g==1, ln_b==0 (checked on the host): a single
                # (x-mu)*rstd per half, with the output DMA overlapping the
                # second half's compute
                for h in range(2):
                    hs = slice(h * 512, (h + 1) * 512)
                    nc.vector.tensor_scalar(o_t[:, hs], x_t[:, hs],
                                            mv[:, 0:1], rstd,
                                            op0=Alu.subtract, op1=Alu.mult)
                    eng = nc.sync if h == 0 else nc.scalar
                    eng.dma_start(out=out_d[sb * 128:(sb + 1) * 128, hs],
                                  in_=o_t[:, hs])
            else:
                y_t = lnp.tile([128, E], F32, name="y_t", tag="y_t")
                nc.vector.scalar_tensor_tensor(
                    y_t, x_t, mv[:, 0:1], g_bc, op0=Alu.subtract, op1=Alu.mult)
                nc.vector.scalar_tensor_tensor(
                    o_t, y_t, rstd, b_bc, op0=Alu.mult, op1=Alu.add)
                nc.sync.dma_start(out=out_d[sb * 128:(sb + 1) * 128, :], in_=o_t)

        # Pipeline: proj(sb+1) is emitted between attention(sb) and
        # outproj(sb) so the PE has work while the softmax-normalize chain
        # (recip -> broadcast -> mul) completes, instead of idling into a
        # low p-state.
        def kv_proj(sb):
            qk_proj(sb, x_ks, wk_sb, kb_sb, k_all)
            v_proj(sb)

        # Attention visit order [1, 2, 3, 0]: the ReduceScatter chain starts
        # after the second-smallest block (~32us) and stays continuous —
        # each rs_in arrives just as the previous collective finishes — and
        # compute ends on the smallest block, so only one RS+LN is exposed
        # in the tail. k/v projections still build incrementally (attention
        # (sb) needs k/v of every block up to sb).
        # proj_rs(sb) is deferred one block (emitted after attention(sb+1))
        # so the softmax-normalize chain of block sb never stalls the PE:
        # attention(sb+1) fills that window.
        kv_proj(0)
        qk_proj(0, x_qs, wq_sb, qb_sb, q_all)
        for sb in range(NS):
            attention(sb)
            if sb >= 1:
                proj_rs(sb - 1)
            if sb + 1 < NS:
                kv_proj(sb + 1)
                qk_proj(sb + 1, x_qs, wq_sb, qb_sb, q_all)
            if sb == NS - 1:
                proj_rs(sb)
            if sb == 2:
                # scheduler-only fences: without them the Tile scheduler
                # hoists the LN chains (whose first op waits on a
                # ReduceScatter) into the middle of the pipeline,
                # head-of-line blocking DVE/SP behind collective waits.
                tc.no_sync_barrier()
                post_ln(0)
        for sb in range(1, NS):
            # one fence per LN block: the scheduler otherwise reorders the
            # rs_out->pchunk DMAs across blocks, head-of-line blocking an
            # already-satisfied LN behind the last collective
            tc.no_sync_barrier()
            post_ln(sb, act_rsqrt=True)

    nc.finalize()
    return nc


def _prep_core(inputs, b, g):
    heads = slice(HL * g, HL * (g + 1))
    query = np.asarray(inputs["query"][b], np.float32)
    key = np.asarray(inputs["key"][b], np.float32)
    value = np.asarray(inputs["value"][b], np.float32)
    Wq_w = np.asarray(inputs["Wq_w"], np.float32)
    Wk_w = np.asarray(inputs["Wk_w"], np.float32)
    Wv_w = np.asarray(inputs["Wv_w"], np.float32)
    Wq_b = np.asarray(inputs["Wq_b"], np.float32)
    Wk_b = np.asarray(inputs["Wk_b"], np.float32)
    Wv_b = np.asarray(inputs["Wv_b"], np.float32)
    out_w = np.asarray(inputs["out_w"], np.float32)
    out_b = np.asarray(inputs["out_b"], np.float32)

    def packb(t):  # [4, 64] -> [128, 2] pair-major
        return np.ascontiguousarray(
            t.reshape(2, 2, Dh).transpose(1, 2, 0).reshape(128, 2))

    def packw(Wh):  # [4, Dh, E] head-major -> [128, NEP*2*256] DR layout
        w = Wh.reshape(256, E).T * WSCALE            # [E, 256]
        w = w.reshape(NEP, 2, 128, 256).transpose(2, 0, 1, 3)
        return np.ascontiguousarray(w.reshape(128, NEP * 2 * 256)).astype(FP8)

    d = {
        "qT": np.ascontiguousarray(query.T).astype(FP8),
        "kT": np.ascontiguousarray(key.T).astype(FP8),
        "vT": np.ascontiguousarray(value.T).astype(FP8),
        "wq": packw(Wq_w[heads]),
        "wk": packw(Wk_w[heads]),
        "wv": packw(Wv_w[heads]),
        "qb": packb(Wq_b[heads]),
        "kb": packb(Wk_b[heads]),
        "wo": np.ascontiguousarray(
            out_w[:, 256 * g:256 * (g + 1)].T * WSCALE).astype(FP8),
        "lng": np.asarray(inputs["ln_g"], np.float32).reshape(1, E).copy(),
        "lnb": np.asarray(inputs["ln_b"], np.float32).reshape(1, E).copy(),
    }
    const = out_b + Wv_b.reshape(E) @ out_w.T
    rows = query.reshape(NS, 4, 128, E)[:, g, :, :].reshape(512, E)
    d["resid"] = np.ascontiguousarray(rows + const[None, :]).astype(BF16)
    return d


def _prep_keep(mask, mult_list, kw, b):
    """Keep matrix packed p-major: [TBLK, nm*kw] bf16."""
    nm = max(1, len(mult_list))
    keep = np.zeros((TBLK, nm, kw), np.float32)
    for mi, (sb, j, b0, b1) in enumerate(mult_list):
        reg = mask[b, sb * SBLK + b0:sb * SBLK + b1,
                   j * TBLK:(j + 1) * TBLK]
        keep[:, mi, 0:b1 - b0] = (~reg).T.astype(np.float32)
    return np.ascontiguousarray(keep.reshape(TBLK, nm * kw)).astype(BF16)


def kernel(**inputs):
    mask = np.asarray(inputs["mask"], bool)
    tiles, mult_list, kw = classify_mask(mask)
    trivial_gb = bool(np.all(np.asarray(inputs["ln_g"]) == 1.0)
                      and np.all(np.asarray(inputs["ln_b"]) == 0.0))
    key_struct = (tiles, mult_list, kw, trivial_gb)
    if key_struct not in _BUILD_CACHE:
        _BUILD_CACHE[key_struct] = build(tiles, mult_list, kw, trivial_gb)
    nc = _BUILD_CACHE[key_struct]

    in_maps = []
    for c in range(N_CORES):
        b, g = c // 4, c % 4
        d = _prep_core(inputs, b, g)
        d["keep"] = _prep_keep(mask, mult_list, kw, b)
        in_maps.append(d)

    res = run_bass_kernel_spmd(nc, in_maps, core_ids=list(range(N_CORES)))

    out = np.empty((B, S, E), np.float32)
    for c in range(N_CORES):
        b, g = c // 4, c % 4
        o = res.results[c]["out"]  # [512, E]
        for sb in range(NS):
            out[b, sb * SBLK + 128 * g: sb * SBLK + 128 * (g + 1), :] = \
                o[sb * 128:(sb + 1) * 128, :]
    return out


# revision 72
# speedup vs baseline: 1.4379x; 1.1028x over previous
"""Distributed MultiHeadAttention + residual + LayerNorm for 8 TRN2 NeuronCores.

Problem: B=2, S=2048, E=1024, H=16, Dh=64 (fp32 in/out).
Sharding: core c = (batch b=c//4, head-group g=c%4); each core computes 4 heads
for its batch. Output projection is row-sharded on the att dim; partials are
combined with per-sblock 4-rank ReduceScatter, then residual+LayerNorm happen
on each core's own row shard.

Single fused pipeline per 512-row sblock:
  [qkv-proj(sb) -> attention(sb) -> (deferred) out-proj(sb-1) -> RS(sb-1)]
The out-projection of block sb is emitted after attention(sb+1) so its
softmax-normalize chain (recip -> partition_broadcast -> mul) never idles the
PE into a low p-state; the ReduceScatter chain runs continuously behind
compute and only the last block's RS + LayerNorm are exposed in the tail.

fp8e4(DoubleRow) matmuls (2 k-tiles per pass = 2x PE throughput) are used for
the QKV projections (x and weights both e4m3; weights pre-scaled by 16 on the
host to stay clear of e4m3 subnormals, de-scaled at PSUM evacuation) and for
the output projection (att and wo both carried at 16x in fp8; both dg halves
contract in one DoubleRow matmul; the 1/256 folds into the evacuation).

Scores are computed transposed (scoresT[t, s]) in bf16 so softmax row sums
come out of the ctx matmul for free via a ones-column appended to V; the
1/sqrt(Dh) scale folds into the exp activation (scale=0.125). The flattened
(dg, tile) loop runs with one tile of lookahead (scores(t+1) before ctx(t))
so PE and the exp-saturated ACT engine pipeline instead of ping-ponging.
Engine placement: exp + proj evacuations on ACT, softmax-normalize + out-proj
evacuations + LN stats on DVE, broadcasts/adds/collectives on Pool (which
cannot touch PSUM on real HW). All bulk loads issue from the SP queue in
need order (an engine whose HWDGE ring fills blocks its own SEQ for >10us,
so ACT/Pool issue no early DMAs). Scheduler-only no_sync_barrier fences pin the LN
chains to the tail so their collective waits never head-of-line block the
pipeline.

Mask handling: tiles are classified on the host from the actual mask input:
fully-masked tiles are skipped, each computed tile gets a column window
[c_lo, 512) excluding fully-masked columns, and a narrow band [b0, b1) where
exp() output is multiplied by a 0/1 keep matrix. The causal structure is
exploited without being hardcoded. LayerNorm takes a fused path when
ln_g==1/ln_b==0 (checked on the host, part of the build cache key).
"""
import sys

if "/opt/trn_rl_repo" not in sys.path:
    sys.path.insert(0, "/opt/trn_rl_repo")

from contextlib import ExitStack

import numpy as np
import ml_dtypes

import concourse.bacc as bacc
import concourse.mybir as mybir
import concourse.tile as tile
from concourse.bass_utils import run_bass_kernel_spmd

BF16 = ml_dtypes.bfloat16
FP8 = ml_dtypes.float8_e4m3fn
F32 = mybir.dt.float32
BF = mybir.dt.bfloat16
F8 = mybir.dt.float8e4

B, S, E, H = 2, 2048, 1024, 16
Dh = E // H
LN_EPS = 1e-5
N_CORES = 8
HL = 4            # local heads per core
GROUPS = [[0, 1, 2, 3], [4, 5, 6, 7]]
NS = 4            # s blocks of 512 (query positions)
SBLK = 512
NT = 16           # t chunks of 128 (key positions)
TBLK = 128
NE = 8            # e chunks of 128 (contraction over E)
NEP = 4           # e chunk pairs (DoubleRow)
WSCALE = 16.0     # host-side weight scale (de-scaled at evacuation)
Act = mybir.ActivationFunctionType
Alu = mybir.AluOpType
DR = mybir.MatmulPerfMode.DoubleRow

_BUILD_CACHE = {}


def _dn128(x):
    return (x // 128) * 128


def _up128(x):
    return -(-x // 128) * 128


def classify_mask(mask):
    """Host-side tile classification from the actual mask input.

    Returns a structure tuple:
      tiles[sb] = list of (j, c_lo, band) with band = (b0, b1, keep_idx) or None
      kw = keep tensor width
    Shared across batches (union), so one SPMD program serves all cores.
    """
    tiles = []
    mult_list = []   # (sb, j, b0, b1)
    for sb in range(NS):
        lst = []
        for j in range(NT):
            # region[b] = mask[b, s-rows, t-cols]; col c of tile = s index
            reg = mask[:, sb * SBLK:(sb + 1) * SBLK, j * TBLK:(j + 1) * TBLK]
            keep_any_col = (~reg).any(axis=2).any(axis=0)     # [SBLK] per s col
            if not keep_any_col.any():
                continue  # fully masked in every batch
            c_lo = _dn128(int(np.argmax(keep_any_col)))
            # prefix [0, c_lo) must be fully masked in all batches (guaranteed
            # since keep_any_col is False there)
            masked_any_col = reg.any(axis=2).any(axis=0)      # [SBLK]
            masked_any_col[:c_lo] = False
            if masked_any_col.any():
                nz = np.nonzero(masked_any_col)[0]
                b0 = max(c_lo, _dn128(int(nz[0])))
                b1 = min(SBLK, _up128(int(nz[-1]) + 1))
                lst.append((j, c_lo, (b0, b1, len(mult_list))))
                mult_list.append((sb, j, b0, b1))
            else:
                lst.append((j, c_lo, None))
        # widest window first so the ctx accumulation group starts with a
        # full-bank clear
        lst.sort(key=lambda t: (t[1], t[0]))
        tiles.append(tuple(lst))
    kw = max([b1 - b0 for (_, _, b0, b1) in mult_list], default=128)
    return tuple(tiles), tuple(mult_list), kw


def build(tiles, mult_list, kw, trivial_gb=False):
    nm = max(1, len(mult_list))
    nc = bacc.Bacc("TRN2", num_devices=N_CORES)

    # ---- I/O -------------------------------------------------------------
    qT_d = nc.dram_tensor("qT", [E, S], F8, kind="ExternalInput")
    kT_d = nc.dram_tensor("kT", [E, S], F8, kind="ExternalInput")
    vT_d = nc.dram_tensor("vT", [E, S], F8, kind="ExternalInput")
    # weights pre-arranged to [128, NEP, 2, 256] (flattened) on the host
    wq_d = nc.dram_tensor("wq", [128, NEP * 2 * 256], F8, kind="ExternalInput")
    wk_d = nc.dram_tensor("wk", [128, NEP * 2 * 256], F8, kind="ExternalInput")
    wv_d = nc.dram_tensor("wv", [128, NEP * 2 * 256], F8, kind="ExternalInput")
    qb_d = nc.dram_tensor("qb", [128, 2], F32, kind="ExternalInput")
    kb_d = nc.dram_tensor("kb", [128, 2], F32, kind="ExternalInput")
    # keep is host-packed p-major ([128, nm*kw]) for 2KB DMA descriptors
    keep_d = nc.dram_tensor("keep", [TBLK, nm * kw], BF, kind="ExternalInput")
    wo_d = nc.dram_tensor("wo", [256, E], F8, kind="ExternalInput")
    resid_d = nc.dram_tensor("resid", [512, E], BF, kind="ExternalInput")
    lng_d = nc.dram_tensor("lng", [1, E], F32, kind="ExternalInput")
    lnb_d = nc.dram_tensor("lnb", [1, E], F32, kind="ExternalInput")
    out_d = nc.dram_tensor("out", [512, E], F32, kind="ExternalOutput")

    rs_in = nc.dram_tensor("rs_in", [S, E], BF, kind="Internal")
    rs_out = nc.dram_tensor("rs_out", [512, E], BF, kind="Internal")

    with tile.TileContext(nc) as tc, ExitStack() as ctx:
        # ---- persistent SBUF tiles --------------------------------------
        persist = ctx.enter_context(tc.tile_pool(name="persist", bufs=1))
        # x split into sb0 / sb1 / sb2+3 tiles so attention(0) can start as
        # soon as the first 1.5MB lands instead of after the full 6MB
        x_qs = [persist.tile([128, NE, SBLK], F8, name="x_q0"),
                persist.tile([128, NE, SBLK], F8, name="x_q1"),
                persist.tile([128, NE, 2 * SBLK], F8, name="x_q23")]
        x_ks = [persist.tile([128, NE, SBLK], F8, name="x_k0"),
                persist.tile([128, NE, SBLK], F8, name="x_k1"),
                persist.tile([128, NE, 2 * SBLK], F8, name="x_k23")]
        x_vs = [persist.tile([128, NE, SBLK], F8, name="x_v0"),
                persist.tile([128, NE, SBLK], F8, name="x_v1"),
                persist.tile([128, NE, 2 * SBLK], F8, name="x_v23")]
        q_all = persist.tile([128, 2, S], BF, name="q_all")   # [d-pair, dg, s]
        k_all = persist.tile([128, 2, S], BF, name="k_all")
        v_all = persist.tile([128, NT, HL, 65], BF, name="v_all")  # [t, j, h, d|1]
        att_sb = persist.tile([128, 2, S], F8, name="att_sb")  # [d-pair, dg, s]
        keep_sb = persist.tile([128, nm, kw], BF, name="keep_sb")
        qb_sb = persist.tile([128, 2], F32, name="qb_sb")
        kb_sb = persist.tile([128, 2], F32, name="kb_sb")
        wq_sb = persist.tile([128, NEP, 2, 256], F8, name="wq_sb")
        wk_sb = persist.tile([128, NEP, 2, 256], F8, name="wk_sb")
        wv_sb = persist.tile([128, NEP, 2, 256], F8, name="wv_sb")
        g_bc = persist.tile([128, E], F32, name="g_bc")
        b_bc = persist.tile([128, E], F32, name="b_bc")
        magic = persist.tile([128, 1], mybir.dt.uint32, name="magic")
        wo_sb = persist.tile([128, 2, E], F8, name="wo_sb")    # [d-pair, dg, e]
        resid_sb = persist.tile([128, NS, E], BF, name="resid_sb")

        nc.vector.memset(magic, 0x5F3759DF)
        # ones column for the row-sum trick
        nc.vector.memset(v_all[:, :, :, 64:65], 1.0)

        # ---- upfront DMA loads --------------------------------------------
        # ALL on the SP queue: ACT must issue no bulk DMAs (a full HWDGE ring
        # blocks its SEQ for >10us, delaying the first evacuations/exps) and
        # SP has no compute until the first rs_in DMAs ~30us in. The DMA
        # engine pool drains FIFO, so issue order == need order: each proj's
        # weight immediately before its x slice (k first: it heads the PE
        # stream), then keep (first band exps), x1, wo (outproj(0)), x23.
        # resid is loaded per-sblock inside proj_rs.
        XCOLS = [slice(0, SBLK), slice(SBLK, 2 * SBLK), slice(2 * SBLK, S)]
        nc.sync.dma_start(out=wk_sb,
                          in_=wk_d.rearrange("p (a b c) -> p a b c", a=NEP, b=2))
        nc.sync.dma_start(out=kb_sb, in_=kb_d[:, :])
        nc.sync.dma_start(
            out=x_ks[0], in_=kT_d[:, XCOLS[0]].rearrange("(c p) s -> p c s", p=128))
        nc.sync.dma_start(out=wq_sb,
                          in_=wq_d.rearrange("p (a b c) -> p a b c", a=NEP, b=2))
        nc.sync.dma_start(out=qb_sb, in_=qb_d[:, :])
        nc.sync.dma_start(
            out=x_qs[0], in_=qT_d[:, XCOLS[0]].rearrange("(c p) s -> p c s", p=128))
        nc.sync.dma_start(out=wv_sb,
                          in_=wv_d.rearrange("p (a b c) -> p a b c", a=NEP, b=2))
        nc.sync.dma_start(
            out=x_vs[0], in_=vT_d[:, XCOLS[0]].rearrange("(c p) s -> p c s", p=128))
        nc.sync.dma_start(out=keep_sb,
                          in_=keep_d.rearrange("p (m s) -> p m s", m=nm))
        for gi in (1, 2):
            nc.sync.dma_start(
                out=x_qs[gi],
                in_=qT_d[:, XCOLS[gi]].rearrange("(c p) s -> p c s", p=128))
            nc.sync.dma_start(
                out=x_ks[gi],
                in_=kT_d[:, XCOLS[gi]].rearrange("(c p) s -> p c s", p=128))
            nc.sync.dma_start(
                out=x_vs[gi],
                in_=vT_d[:, XCOLS[gi]].rearrange("(c p) s -> p c s", p=128))
            if gi == 1:
                nc.sync.dma_start(out=wo_sb,
                                  in_=wo_d.rearrange("(c p) d -> p c d", p=128))

        # ---- pools -------------------------------------------------------
        pp_pool = ctx.enter_context(tc.tile_pool(name="pp", bufs=2, space="PSUM"))
        sc_pool = ctx.enter_context(tc.tile_pool(name="sc", bufs=2, space="PSUM"))
        ctx_pool = ctx.enter_context(tc.tile_pool(name="ctxp", bufs=2, space="PSUM"))
        probs_pool = ctx.enter_context(tc.tile_pool(name="probs", bufs=6))
        small = ctx.enter_context(tc.tile_pool(name="small", bufs=4))
        work = ctx.enter_context(tc.tile_pool(name="work", bufs=4))
        lnp = ctx.enter_context(tc.tile_pool(name="lnp", bufs=2))

        INV_W = 1.0 / WSCALE

        def xgrp(sb):
            # (tile-group index, column offset within the group) for sblock
            return (sb, 0) if sb < 2 else (2, (sb - 2) * SBLK)

        def qk_proj(sb, xts, w_sb, bias_sb, dst):
            gi, off = xgrp(sb)
            xs = slice(off, off + SBLK)
            ss = slice(sb * SBLK, (sb + 1) * SBLK)
            for dg in range(2):
                ps = pp_pool.tile([128, SBLK], F32, name="ps", tag="pp")
                for ep in range(NEP):
                    nc.tensor.matmul(
                        ps,
                        w_sb[:, ep, :, dg * 128:(dg + 1) * 128],
                        xts[gi][:, 2 * ep:2 * ep + 2, xs],
                        start=(ep == 0), stop=(ep == NEP - 1),
                        perf_mode=DR,
                    )
                # evacuation on ACT: fused 1/WSCALE descale + bias add
                nc.scalar.activation(dst[:, dg, ss], ps, Act.Identity,
                                     bias=bias_sb[:, dg:dg + 1], scale=INV_W)

        def v_proj(sb):
            gi, off = xgrp(sb)
            # v[t, (h d)] for the 4 t-tiles of this sblock, two t-tiles per
            # PSUM bank
            for jj in range(2):
                j0 = 4 * sb + 2 * jj
                psv = pp_pool.tile([128, SBLK], F32, name="ps", tag="pp")
                for tl in range(2):
                    toff = off + (2 * jj + tl) * TBLK
                    ts = slice(toff, toff + TBLK)
                    for ep in range(NEP):
                        nc.tensor.matmul(
                            psv[:, tl * 256:(tl + 1) * 256],
                            x_vs[gi][:, 2 * ep:2 * ep + 2, ts],
                            wv_sb[:, ep, :, :],
                            start=(ep == 0), stop=(ep == NEP - 1),
                            perf_mode=DR,
                        )
                # v gets an extra x16 (cancelling 1/WSCALE exactly): att_sb
                # is carried at 16x so its fp8 quantization error is relative,
                # and the 1/(16*16) is folded into the outproj evacuation
                nc.scalar.activation(
                    v_all[:, j0:j0 + 2, :, 0:64],
                    psv.rearrange("p (j h d) -> p j h d", j=2, h=HL),
                    Act.Identity, scale=1.0)

        def attention(sb):
            tlist = tiles[sb]

            def normalize(dg, cA, cB):
                # recip from an SBUF copy of the PSUM ones-row (the custom-DVE
                # recip op reads garbage from PSUM on real HW), broadcast on
                # Pool; this chain gates outproj. For the LAST block (whose
                # chain is fully exposed in the tail) the sums copies run in
                # parallel on ACT (idle after the final exp) and DVE, and the
                # normalize muls split into column chunks so the outproj
                # chunks emitted right after can pipeline with them.
                last_blk = sb == NS - 1 and dg == 1
                for i, cx in ((0, cA), (1, cB)):
                    sums = small.tile([1, SBLK], F32, name="sums", tag="sums")
                    ceng = nc.scalar if (last_blk and i == 0) else nc.vector
                    if ceng is nc.scalar:
                        ceng.activation(sums, cx[64:65, :], Act.Identity)
                    else:
                        ceng.tensor_copy(sums, cx[64:65, :])
                    recip = small.tile([1, SBLK], F32, name="recip", tag="recip")
                    nc.vector.reciprocal_approx_fast(recip, sums)
                    bc = work.tile([64, SBLK], F32, name="bc", tag="bc")
                    nc.gpsimd.partition_broadcast(bc, recip, channels=64)
                    ncols = 4 if last_blk else 1
                    for cchunk in range(ncols):
                        colw = SBLK // ncols
                        c0 = cchunk * colw
                        nc.vector.tensor_mul(
                            att_sb[64 * i:64 * i + 64, dg,
                                   sb * SBLK + c0:sb * SBLK + c0 + colw],
                            cx[0:64, c0:c0 + colw], bc[:, c0:c0 + colw])

            # single flattened (dg, tile) stream with one tile of lookahead
            # ACROSS the dg boundary so ACT never drains mid-block
            cAB = {}
            pend = None

            def emit_ctx(p):
                _dg, _j, _cs, _p2, _first, _last = p
                cA, cB = cAB[_dg]
                nc.tensor.matmul(cA[:, _cs], v_all[:, _j, 2 * _dg, :],
                                 _p2[:, 0, _cs], start=_first, stop=_last)
                nc.tensor.matmul(cB[:, _cs], v_all[:, _j, 2 * _dg + 1, :],
                                 _p2[:, 1, _cs], start=_first, stop=_last)
                if _last:
                    normalize(_dg, cA, cB)

            for dg in range(2):
                for idx, (j, c_lo, band) in enumerate(tlist):
                    first, last = idx == 0, idx == len(tlist) - 1
                    if first:
                        cAB[dg] = (
                            ctx_pool.tile([65, SBLK], F32, name="cA", tag="ctx"),
                            ctx_pool.tile([65, SBLK], F32, name="cB", tag="ctx"))
                    # two-bank scores tile: head A in half 0, head B in half 1
                    sc2 = sc_pool.tile([128, 2, SBLK], F32, name="sc2", tag="sc")
                    ts = slice(j * TBLK, (j + 1) * TBLK)
                    ss = slice(sb * SBLK + c_lo, (sb + 1) * SBLK)
                    cs = slice(c_lo, SBLK)
                    nc.tensor.matmul(sc2[:, 0, cs], k_all[0:64, dg, ts],
                                     q_all[0:64, dg, ss], start=True, stop=True)
                    nc.tensor.matmul(sc2[:, 1, cs], k_all[64:128, dg, ts],
                                     q_all[64:128, dg, ss], start=True, stop=True)
                    p2 = probs_pool.tile([128, 2, SBLK], BF, name="p2", tag="pr")
                    nc.scalar.activation(p2[:, :, cs], sc2[:, :, cs], Act.Exp,
                                         scale=0.125)
                    if band is not None:
                        b0, b1, mi = band
                        bs_ = slice(b0, b1)
                        keep_b = keep_sb[:, mi:mi + 1, 0:b1 - b0].to_broadcast(
                            [128, 2, b1 - b0])
                        nc.vector.tensor_mul(p2[:, :, bs_], p2[:, :, bs_], keep_b)
                    if pend is not None:
                        emit_ctx(pend)
                    pend = (dg, j, cs, p2, first, last)
            emit_ctx(pend)

        def proj_rs(sb):
            # out projection partials + ReduceScatter for this sblock
            if sb == 1 and not trivial_gb:
                # LN consts, needed first at post_ln(0) (emitted two sblocks
                # later): issued here to stay clear of the x-chunk burst
                nc.gpsimd.dma_start(out=g_bc, in_=lng_d[0:1, :].to_broadcast([128, E]))
                nc.gpsimd.dma_start(out=b_bc, in_=lnb_d[0:1, :].to_broadcast([128, E]))
            # residual rows for this sblock's LN, prefetched well before use
            nc.sync.dma_start(out=resid_sb[:, sb, :],
                              in_=resid_d[sb * 128:(sb + 1) * 128, :])
            for sc in range(4):   # 128-row chunks within sblock
                srow = sb * 4 + sc
                row = slice(srow * 128, (srow + 1) * 128)
                pcopy = work.tile([128, E], BF, name="pcopy", tag="pcopy")
                for eo in range(2):
                    pp = pp_pool.tile([128, SBLK], F32, name="ps", tag="pp")
                    # both dg halves contract in ONE DoubleRow fp8 matmul
                    nc.tensor.matmul(
                        pp,
                        att_sb[:, :, row],
                        wo_sb[:, :, eo * SBLK:(eo + 1) * SBLK],
                        start=True, stop=True,
                        perf_mode=DR,
                    )
                    # 1/256 undoes the x16 on att_sb and the x16 on wo.
                    # For the last sblock ACT is past its final exp, so
                    # splitting the evacuations ACT/DVE halves the stagger
                    # on the rs_in(3) critical path.
                    if sb == NS - 1 and eo == 0:
                        nc.scalar.activation(pcopy[:, 0:SBLK], pp,
                                             Act.Identity, scale=1.0 / 256.0)
                    else:
                        nc.vector.tensor_scalar_mul(
                            pcopy[:, eo * SBLK:(eo + 1) * SBLK], pp, 1.0 / 256.0)
                nc.sync.dma_start(out=rs_in[row, :], in_=pcopy)
            nc.gpsimd.collective_compute(
                "ReduceScatter", Alu.add,
                ins=[rs_in[sb * SBLK:(sb + 1) * SBLK, :]],
                outs=[rs_out[sb * 128:(sb + 1) * 128, :]],
                replica_groups=GROUPS,
            )

        def post_ln(sb, act_rsqrt=False):
            # residual + LN on own 128 rows of this sblock (runs one sblock
            # behind the RS so its waits never head-of-line block the queues)
            pchunk = lnp.tile([128, E], BF, name="pchunk", tag="pchunk")
            x_t = lnp.tile([128, E], F32, name="x_t", tag="x_t")
            stats = small.tile([128, 2, 6], F32, name="stats", tag="stats")
            for h in range(2):
                hs = slice(h * 512, (h + 1) * 512)
                deng = nc.sync if h == 0 else nc.scalar
                deng.dma_start(out=pchunk[:, hs],
                               in_=rs_out[sb * 128:(sb + 1) * 128, hs])
                eng = nc.gpsimd if h == 0 else nc.vector
                eng.tensor_add(x_t[:, hs], resid_sb[:, sb, hs], pchunk[:, hs])
                nc.vector.bn_stats(stats[:, h, :], x_t[:, hs])
            mv = small.tile([128, 2], F32, name="mv", tag="mv")
            nc.vector.bn_aggr(mv, stats)
            # rstd = rsqrt(var + eps) on DVE (bit-trick seed + 2 Newton iters)
            # so ACT never leaves the exp table set
            U32 = mybir.dt.uint32
            ws = small.tile([128, 1], F32, name="ws", tag="ws")
            nc.vector.tensor_scalar_add(ws, mv[:, 1:2], LN_EPS)
            rstd = small.tile([128, 1], F32, name="rstd", tag="rstd")
            if act_rsqrt:
                # tail LNs run after the last exp: the ACT table switch is
                # free to take there; Sqrt+reciprocal replaces the Newton
                # chain (Rsqrt itself is blocked for accuracy)
                sq = small.tile([128, 1], F32, name="sq", tag="sq")
                nc.scalar.activation(sq, ws, Act.Sqrt)
                nc.vector.reciprocal(rstd, sq)
            else:
                hbits = small.tile([128, 1], U32, name="hbits", tag="hbits")
                nc.vector.tensor_scalar(hbits, ws.bitcast(U32), 1, None,
                                        op0=Alu.logical_shift_right)
                nc.vector.scalar_tensor_tensor(
                    rstd.bitcast(U32), magic, 0, hbits, op0=Alu.bypass,
                    op1=Alu.subtract)
                nt = small.tile([128, 1], F32, name="nt", tag="nt")
                for _ in range(2):
                    nc.vector.tensor_mul(nt, ws, rstd)
                    nc.vector.tensor_mul(nt, nt, rstd)
                    nc.vector.tensor_scalar(nt, nt, -0.5, 1.5, op0=Alu.mult,
                                            op1=Alu.add)
                    nc.vector.tensor_mul(rstd, rstd, nt)
            o_t = lnp.tile([128, E], F32, name="o_t", tag="o_t")
            if trivial_gb:
                # ln_g==1, ln_b==0 (checked on the host): a single
                # (x-mu)*rstd per half, with the output DMA overlapping the
                # second half's compute
                for h in range(2):
                    hs = slice(h * 512, (h + 1) * 512)
                    nc.vector.tensor_scalar(o_t[:, hs], x_t[:, hs],
                                            mv[:, 0:1], rstd,
                                            op0=Alu.subtract, op1=Alu.mult)
                    eng = nc.sync if h == 0 else nc.scalar
                    eng.dma_start(out=out_d[sb * 128:(sb + 1) * 128, hs],
                                  in_=o_t[:, hs])
            else:
                y_t = lnp.tile([128, E], F32, name="y_t", tag="y_t")
                nc.vector.scalar_tensor_tensor(
                    y_t, x_t, mv[:, 0:1], g_bc, op0=Alu.subtract, op1=Alu.mult)
                nc.vector.scalar_tensor_tensor(
                    o_t, y_t, rstd, b_bc, op0=Alu.mult, op1=Alu.add)
                nc.sync.dma_start(out=out_d[sb * 128:(sb + 1) * 128, :], in_=o_t)

        # Pipeline: proj(sb+1) is emitted between attention(sb) and
        # outproj(sb) so the PE has work while the softmax-normalize chain
        # (recip -> broadcast -> mul) completes, instead of idling into a
        # low p-state.
        def kv_proj(sb):
            qk_proj(sb, x_ks, wk_sb, kb_sb, k_all)
            v_proj(sb)

        # Attention visit order [1, 2, 3, 0]: the ReduceScatter chain starts
        # after the second-smallest block (~32us) and stays continuous —
        # each rs_in arrives just as the previous collective finishes — and
        # compute ends on the smallest block, so only one RS+LN is exposed
        # in the tail. k/v projections still build incrementally (attention
        # (sb) needs k/v of every block up to sb).
        # proj_rs(sb) is deferred one block (emitted after attention(sb+1))
        # so the softmax-normalize chain of block sb never stalls the PE:
        # attention(sb+1) fills that window.
        kv_proj(0)
        qk_proj(0, x_qs, wq_sb, qb_sb, q_all)
        for sb in range(NS):
            attention(sb)
            if sb >= 1:
                proj_rs(sb - 1)
            if sb + 1 < NS:
                kv_proj(sb + 1)
                qk_proj(sb + 1, x_qs, wq_sb, qb_sb, q_all)
            if sb == NS - 1:
                proj_rs(sb)
            if sb == 2:
                # scheduler-only fences: without them the Tile scheduler
                # hoists the LN chains (whose first op waits on a
                # ReduceScatter) into the middle of the pipeline,
                # head-of-line blocking DVE/SP behind collective waits.
                tc.no_sync_barrier()
                post_ln(0)
        for sb in range(1, NS):
            # one fence per LN block: the scheduler otherwise reorders the
            # rs_out->pchunk DMAs across blocks, head-of-line blocking an
            # already-satisfied LN behind the last collective
            tc.no_sync_barrier()
            post_ln(sb, act_rsqrt=True)

    nc.finalize()
    return nc


def _prep_core(inputs, b, g):
    heads = slice(HL * g, HL * (g + 1))
    query = np.asarray(inputs["query"][b], np.float32)
    key = np.asarray(inputs["key"][b], np.float32)
    value = np.asarray(inputs["value"][b], np.float32)
    Wq_w = np.asarray(inputs["Wq_w"], np.float32)
    Wk_w = np.asarray(inputs["Wk_w"], np.float32)
    Wv_w = np.asarray(inputs["Wv_w"], np.float32)
    Wq_b = np.asarray(inputs["Wq_b"], np.float32)
    Wk_b = np.asarray(inputs["Wk_b"], np.float32)
    Wv_b = np.asarray(inputs["Wv_b"], np.float32)
    out_w = np.asarray(inputs["out_w"], np.float32)
    out_b = np.asarray(inputs["out_b"], np.float32)

    def packb(t):  # [4, 64] -> [128, 2] pair-major
        return np.ascontiguousarray(
            t.reshape(2, 2, Dh).transpose(1, 2, 0).reshape(128, 2))

    def packw(Wh):  # [4, Dh, E] head-major -> [128, NEP*2*256] DR layout
        w = Wh.reshape(256, E).T * WSCALE            # [E, 256]
        w = w.reshape(NEP, 2, 128, 256).transpose(2, 0, 1, 3)
        return np.ascontiguousarray(w.reshape(128, NEP * 2 * 256)).astype(FP8)

    d = {
        "qT": np.ascontiguousarray(query.T).astype(FP8),
        "kT": np.ascontiguousarray(key.T).astype(FP8),
        "vT": np.ascontiguousarray(value.T).astype(FP8),
        "wq": packw(Wq_w[heads]),
        "wk": packw(Wk_w[heads]),
        "wv": packw(Wv_w[heads]),
        "qb": packb(Wq_b[heads]),
        "kb": packb(Wk_b[heads]),
        "wo": np.ascontiguousarray(
            out_w[:, 256 * g:256 * (g + 1)].T * WSCALE).astype(FP8),
        "lng": np.asarray(inputs["ln_g"], np.float32).reshape(1, E).copy(),
        "lnb": np.asarray(inputs["ln_b"], np.float32).reshape(1, E).copy(),
    }
    const = out_b + Wv_b.reshape(E) @ out_w.T
    rows = query.reshape(NS, 4, 128, E)[:, g, :, :].reshape(512, E)
    d["resid"] = np.ascontiguousarray(rows + const[None, :]).astype(BF16)
    return d


def _prep_keep(mask, mult_list, kw, b):
    """Keep matrix packed p-major: [TBLK, nm*kw] bf16."""
    nm = max(1, len(mult_list))
    keep = np.zeros((TBLK, nm, kw), np.float32)
    for mi, (sb, j, b0, b1) in enumerate(mult_list):
        reg = mask[b, sb * SBLK + b0:sb * SBLK + b1,
                   j * TBLK:(j + 1) * TBLK]
        keep[:, mi, 0:b1 - b0] = (~reg).T.astype(np.float32)
    return np.ascontiguousarray(keep.reshape(TBLK, nm * kw)).astype(BF16)


def kernel(**inputs):
    mask = np.asarray(inputs["mask"], bool)
    tiles, mult_list, kw = classify_mask(mask)
    trivial_gb = bool(np.all(np.asarray(inputs["ln_g"]) == 1.0)
                      and np.all(np.asarray(inputs["ln_b"]) == 0.0))
    key_struct = (tiles, mult_list, kw, trivial_gb)
    if key_struct not in _BUILD_CACHE:
        _BUILD_CACHE[key_struct] = build(tiles, mult_list, kw, trivial_gb)
    nc = _BUILD_CACHE[key_struct]

    in_maps = []
    for c in range(N_CORES):
        b, g = c // 4, c % 4
        d = _prep_core(inputs, b, g)
        d["keep"] = _prep_keep(mask, mult_list, kw, b)
        in_maps.append(d)

    res = run_bass_kernel_spmd(nc, in_maps, core_ids=list(range(N_CORES)))

    out = np.empty((B, S, E), np.float32)
    for c in range(N_CORES):
        b, g = c // 4, c % 4
        o = res.results[c]["out"]  # [512, E]
        for sb in range(NS):
            out[b, sb * SBLK + 128 * g: sb * SBLK + 128 * (g + 1), :] = \
                o[sb * 128:(sb + 1) * 128, :]
    return out
